# revision 1
# baseline (speedup 1.0000x reference)
"""GPT decoder (V=32000,S=1024,D=768,H=12,HID=3072,L=4,B=2) on 8 trn2 cores.

Sharding: sequence-parallel body — core c owns tokens [256c, 256c+256) of the
flattened [2048] token stream (cores 0-3 = batch 0, cores 4-7 = batch 1).
Per layer, each core computes qkv for its tokens, K/V are exchanged with an
AllGather inside each 4-core batch group, attention/FFN stay local.  The tied
lm_head runs per-core over the full vocab for the local 256 tokens.
Matmuls in bf16 with fp32 PSUM accumulation; activations/norms in fp32.
Activations are feature-major [D, tok] so the contraction dim is on partitions.
"""
import math

import ml_dtypes
import numpy as np

import concourse.bass as bass
import concourse.mybir as mybir
import concourse.tile as tile
from concourse import bacc
from concourse.bass_utils import run_bass_kernel_spmd

F32 = mybir.dt.float32
BF16 = mybir.dt.bfloat16
AF = mybir.ActivationFunctionType
ALU = mybir.AluOpType

N_CORES = 8
GROUPS = [[0, 1, 2, 3], [4, 5, 6, 7]]
V, S, D, H, HID, L, B = 32000, 1024, 768, 12, 3072, 4, 2
HD = D // H          # 64
TOK = 256            # tokens per core
NK = D // 128        # 6 feature chunks
NM_QKV = 3 * D // 128   # 18
NM_HID = HID // 128     # 24
EPS = 1e-5
VB = 500             # lm_head vocab block
NVB = V // VB        # 64

TRACE = False
LAST_RESULT = None

_NC_CACHE = None


def _ln(nc, tc, pools, x_fm, g_ap, b_ap, out_bf):
    """LayerNorm over features (partition dim) via ones-matmul reductions.

    x_fm: [128, NK, 256] f32 sbuf.  g_ap/b_ap: [128,1] per-chunk slices fn.
    out_bf: [128, NK, 256] bf16 sbuf tile to fill with gamma*x_hat+beta.
    """
    ps_stat, ps_mm, tmp, stt = pools["ps_stat"], pools["ps_mm"], pools["tmp"], pools["stt"]
    ones_bf = pools["ones_bf"]
    ones_row = pools["ones_row"]

    s1 = ps_stat.tile([1, TOK], F32, tag="lnstat")
    s2 = ps_stat.tile([1, TOK], F32, tag="lnstat")
    for k in range(NK):
        xb = tmp.tile([128, TOK], BF16, tag="lnxb")
        nc.vector.tensor_copy(xb[:], x_fm[:, k, :])
        nc.tensor.matmul(s1[:], ones_bf[:], xb[:], start=(k == 0), stop=(k == NK - 1))
        sq = tmp.tile([128, TOK], BF16, tag="lnsq")
        nc.vector.tensor_mul(sq[:], xb[:], xb[:])
        nc.tensor.matmul(s2[:], ones_bf[:], sq[:], start=(k == 0), stop=(k == NK - 1))

    mean = stt.tile([1, TOK], F32, tag="mean")
    nc.vector.tensor_scalar_mul(mean[:], s1[:], 1.0 / D)
    var = stt.tile([1, TOK], F32, tag="var")
    nc.vector.tensor_scalar_mul(var[:], s2[:], 1.0 / D)
    msq = stt.tile([1, TOK], F32, tag="msq")
    nc.vector.tensor_mul(msq[:], mean[:], mean[:])
    nc.vector.tensor_sub(var[:], var[:], msq[:])
    nc.vector.tensor_scalar_add(var[:], var[:], EPS)
    rec = stt.tile([1, TOK], F32, tag="rec")
    nc.vector.reciprocal(rec[:], var[:])
    a = stt.tile([1, TOK], F32, tag="a")
    nc.scalar.sqrt(a[:], rec[:])                      # rstd = sqrt(1/(var+eps))
    colb = stt.tile([1, TOK], F32, tag="colb")
    nc.vector.tensor_mul(colb[:], mean[:], a[:])
    nc.vector.tensor_scalar_mul(colb[:], colb[:], -1.0)  # -mean*rstd

    ba = ps_mm.tile([128, TOK], F32, tag="mm")
    nc.tensor.matmul(ba[:], ones_row[:], a[:], start=True, stop=True)
    bb = ps_mm.tile([128, TOK], F32, tag="mm")
    nc.tensor.matmul(bb[:], ones_row[:], colb[:], start=True, stop=True)

    for k in range(NK):
        t = tmp.tile([128, TOK], F32, tag="lnt")
        nc.vector.tensor_mul(t[:], x_fm[:, k, :], ba[:])
        nc.vector.tensor_add(t[:], t[:], bb[:])
        nc.scalar.activation(out_bf[:, k, :], t[:], AF.Identity,
                             bias=b_ap[k], scale=g_ap[k])


def build_nc():
    nc = bacc.Bacc("TRN2", target_bir_lowering=False, debug=False,
                   enable_asserts=True, num_devices=N_CORES)

    d_x0 = nc.dram_tensor("x0", [NK, 128, TOK], F32, kind="ExternalInput")
    d_mask = nc.dram_tensor("mask", [8, 128, TOK], F32, kind="ExternalInput")
    d_qkvw = nc.dram_tensor("qkvw", [L, NK, 128, 3 * D], BF16, kind="ExternalInput")
    d_qkvb = nc.dram_tensor("qkvb", [L, 128, NM_QKV], F32, kind="ExternalInput")
    d_projw = nc.dram_tensor("projw", [L, NK, 128, D], BF16, kind="ExternalInput")
    d_projb = nc.dram_tensor("projb", [L, 128, NK], F32, kind="ExternalInput")
    d_f1w = nc.dram_tensor("f1w", [L, NK, 128, HID], BF16, kind="ExternalInput")
    d_f1b = nc.dram_tensor("f1b", [L, 128, NM_HID], F32, kind="ExternalInput")
    d_f2w = nc.dram_tensor("f2w", [L, NM_HID, 128, D], BF16, kind="ExternalInput")
    d_f2b = nc.dram_tensor("f2b", [L, 128, NK], F32, kind="ExternalInput")
    d_n1g = nc.dram_tensor("n1g", [L, 128, NK], F32, kind="ExternalInput")
    d_n1b = nc.dram_tensor("n1b", [L, 128, NK], F32, kind="ExternalInput")
    d_n2g = nc.dram_tensor("n2g", [L, 128, NK], F32, kind="ExternalInput")
    d_n2b = nc.dram_tensor("n2b", [L, 128, NK], F32, kind="ExternalInput")
    d_fing = nc.dram_tensor("fing", [128, NK], F32, kind="ExternalInput")
    d_finb = nc.dram_tensor("finb", [128, NK], F32, kind="ExternalInput")
    d_wemb = nc.dram_tensor("wemb", [NVB, NK, 128, VB], BF16, kind="ExternalInput")
    d_out = nc.dram_tensor("logits", [TOK, V], F32, kind="ExternalOutput")

    with tile.TileContext(nc) as tc:
        from contextlib import ExitStack
        with ExitStack() as ctx:
            const = ctx.enter_context(tc.tile_pool(name="const", bufs=1))
            res = ctx.enter_context(tc.tile_pool(name="res", bufs=1))
            tmp = ctx.enter_context(tc.tile_pool(name="tmp", bufs=3))
            stt = ctx.enter_context(tc.tile_pool(name="stt", bufs=2))
            attn = ctx.enter_context(tc.tile_pool(name="attn", bufs=2))
            lmo = ctx.enter_context(tc.tile_pool(name="lmo", bufs=3))
            ps_mm = ctx.enter_context(tc.tile_pool(name="ps_mm", bufs=2, space="PSUM"))
            ps_sc = ctx.enter_context(tc.tile_pool(name="ps_sc", bufs=2, space="PSUM"))
            ps_tr = ctx.enter_context(tc.tile_pool(name="ps_tr", bufs=2, space="PSUM"))
            ps_stat = ctx.enter_context(tc.tile_pool(name="ps_stat", bufs=2, space="PSUM"))
            dram = ctx.enter_context(tc.tile_pool(name="dram", bufs=2, space="DRAM"))

            identity = const.tile([128, 128], BF16)
            from concourse.masks import make_identity
            make_identity(nc, identity[:])
            ones_bf = const.tile([128, 1], BF16)
            nc.any.memset(ones_bf[:], 1.0)
            ones_row = const.tile([1, 128], F32)
            nc.any.memset(ones_row[:], 1.0)
            ones2 = const.tile([128, 128], F32)
            nc.any.memset(ones2[:], 1.0)

            # Residual stream + mask + params, resident in SBUF
            x_fm = res.tile([128, NK, TOK], F32)
            for k in range(NK):
                nc.sync.dma_start(x_fm[:, k, :], d_x0.ap()[k])
            mask_t = res.tile([128, 8, TOK], F32)
            for t in range(8):
                nc.sync.dma_start(mask_t[:, t, :], d_mask.ap()[t])
            qkvb_a = res.tile([128, L, NM_QKV], F32)
            projb_a = res.tile([128, L, NK], F32)
            f1b_a = res.tile([128, L, NM_HID], F32)
            f2b_a = res.tile([128, L, NK], F32)
            n1g_a = res.tile([128, L, NK], F32)
            n1b_a = res.tile([128, L, NK], F32)
            n2g_a = res.tile([128, L, NK], F32)
            n2b_a = res.tile([128, L, NK], F32)
            fing_a = res.tile([128, NK], F32)
            finb_a = res.tile([128, NK], F32)
            for l in range(L):
                nc.sync.dma_start(qkvb_a[:, l, :], d_qkvb.ap()[l])
                nc.sync.dma_start(projb_a[:, l, :], d_projb.ap()[l])
                nc.sync.dma_start(f1b_a[:, l, :], d_f1b.ap()[l])
                nc.sync.dma_start(f2b_a[:, l, :], d_f2b.ap()[l])
                nc.sync.dma_start(n1g_a[:, l, :], d_n1g.ap()[l])
                nc.sync.dma_start(n1b_a[:, l, :], d_n1b.ap()[l])
                nc.sync.dma_start(n2g_a[:, l, :], d_n2g.ap()[l])
                nc.sync.dma_start(n2b_a[:, l, :], d_n2b.ap()[l])
            nc.sync.dma_start(fing_a[:], d_fing.ap())
            nc.sync.dma_start(finb_a[:], d_finb.ap())

            pools = dict(ps_stat=ps_stat, ps_mm=ps_mm, tmp=tmp, stt=stt,
                         ones_bf=ones_bf, ones_row=ones_row)

            h_bf = res.tile([128, NK, TOK], BF16)
            qkv_sb = res.tile([128, NM_QKV, TOK], BF16)
            # v_own: per head 66 cols = [onesA | v(64) | onesB]; even heads use
            # cols [1:66] (ones last -> den row 64), odd heads cols [0:65]
            # (ones first -> den row 63) so AV lands vals at the head's rows.
            v_own = res.tile([128, 2, H, 66], BF16)
            nc.any.memset(v_own[:, :, :, 0:1], 1.0)
            nc.any.memset(v_own[:, :, :, 65:66], 1.0)
            k_all = res.tile([128, NK, 4 * TOK], BF16)
            v_all = res.tile([128, 8, H * 66], BF16)
            vals_fm = res.tile([128, NK, TOK], BF16)
            h2_sb = res.tile([128, NM_HID, TOK], BF16)

            def qslice(h, qt):
                return qkv_sb[(h % 2) * 64:(h % 2) * 64 + 64, h // 2,
                              128 * qt:128 * qt + 128]

            for l in range(L):
                gs = [n1g_a[:, l, k:k + 1] for k in range(NK)]
                bs = [n1b_a[:, l, k:k + 1] for k in range(NK)]
                _ln(nc, tc, pools, x_fm, gs, bs, h_bf)

                # qkv
                with tc.tile_pool(name="wqkv", bufs=NK) as pw:
                    wk = []
                    for k in range(NK):
                        w = pw.tile([128, 3 * D], BF16, tag="w")
                        nc.sync.dma_start(w[:], d_qkvw.ap()[l, k])
                        wk.append(w)
                    for m in range(NM_QKV):
                        ps = ps_mm.tile([128, TOK], F32, tag="mm")
                        for k in range(NK):
                            nc.tensor.matmul(ps[:], wk[k][:, 128 * m:128 * (m + 1)],
                                             h_bf[:, k, :],
                                             start=(k == 0), stop=(k == NK - 1))
                        scale = 1.0 / math.sqrt(HD) if m < NK else 1.0
                        nc.scalar.activation(qkv_sb[:, m, :], ps[:], AF.Identity,
                                             bias=qkvb_a[:, l, m:m + 1], scale=scale)

                # own-chunk v -> token-major
                for h in range(H):
                    o = (h % 2) * 64
                    for t in range(2):
                        src = qkv_sb[o:o + 64, 12 + h // 2, 128 * t:128 * (t + 1)]
                        pt = ps_tr.tile([128, 64], BF16, tag="tr")
                        nc.tensor.transpose(pt[:], src,
                                            identity[o:o + 64, o:o + 64])
                        nc.vector.tensor_copy(v_own[:, t, h, 1:65], pt[:])

                # KV exchange within batch group: slots 0-5 = k chunks (256 of
                # 264 cols), slots 6-11 = v_own (2 tok-chunks x 3 blocks of
                # 4 heads x 66).
                b_in = dram.tile([12, 128, 264], BF16, tag="bin")
                b_out = dram.tile([48, 128, 264], BF16, tag="bout")
                for k in range(NK):
                    nc.sync.dma_start(b_in[k, :, 0:TOK], qkv_sb[:, NK + k, :])
                for t in range(2):
                    for j in range(3):
                        nc.sync.dma_start(b_in[6 + 3 * t + j],
                                          v_own[:, t, 4 * j:4 * (j + 1), :])
                nc.gpsimd.collective_compute(
                    "AllGather", ALU.bypass, replica_groups=GROUPS,
                    ins=[b_in.opt()], outs=[b_out.opt()])
                for c in range(4):
                    for k in range(NK):
                        nc.sync.dma_start(k_all[:, k, TOK * c:TOK * (c + 1)],
                                          b_out[12 * c + k, :, 0:TOK])
                    for t in range(2):
                        for j in range(3):
                            nc.sync.dma_start(
                                v_all[:, 2 * c + t, 264 * j:264 * (j + 1)],
                                b_out[12 * c + 6 + 3 * t + j])

                # attention: S^T per kt-chunk, exp, AV with ones-col -> den row
                for h in range(H):
                    o = (h % 2) * 64
                    kslc = slice(o, o + 64)
                    av = ps_mm.tile([128, TOK], F32, tag="mm")
                    dn = ps_stat.tile([1, TOK], F32, tag="lnstat")
                    vcol = 66 * h + 1
                    for kc in range(8):
                        st = ps_sc.tile([128, TOK], F32, tag="sc")
                        nc.tensor.matmul(
                            st[:],
                            k_all[kslc, h // 2, 128 * kc:128 * (kc + 1)],
                            qkv_sb[kslc, h // 2, :],
                            start=True, stop=True)
                        nc.vector.tensor_add(st[:], st[:], mask_t[:, kc, :])
                        pt_t = attn.tile([128, TOK], BF16, tag="ptsb")
                        nc.scalar.activation(pt_t[:], st[:], AF.Exp)
                        nc.tensor.matmul(av[o:o + 64, :],
                                         v_all[:, kc, vcol:vcol + 64],
                                         pt_t[:],
                                         start=(kc == 0), stop=(kc == 7))
                        nc.tensor.matmul(dn[:], ones_bf[:], pt_t[:],
                                         start=(kc == 0), stop=(kc == 7))
                    rden = stt.tile([1, TOK], F32, tag="rden")
                    nc.vector.reciprocal(rden[:], dn[:])
                    bc = ps_tr.tile([128, TOK], F32, tag="tr")
                    nc.tensor.matmul(bc[o:o + 64, :], ones2[0:1, 0:64],
                                     rden[:], start=True, stop=True)
                    bcs = tmp.tile([128, TOK], F32, tag="lnt")
                    nc.scalar.copy(bcs[o:o + 64, :], bc[o:o + 64, :])
                    nc.vector.tensor_mul(vals_fm[o:o + 64, h // 2, :],
                                         av[o:o + 64, :], bcs[o:o + 64, :])

                # proj + residual
                with tc.tile_pool(name="wproj", bufs=NK) as pw:
                    pk = []
                    for k in range(NK):
                        w = pw.tile([128, D], BF16, tag="w")
                        nc.sync.dma_start(w[:], d_projw.ap()[l, k])
                        pk.append(w)
                    for m in range(NK):
                        ps = ps_mm.tile([128, TOK], F32, tag="mm")
                        for k in range(NK):
                            nc.tensor.matmul(ps[:], pk[k][:, 128 * m:128 * (m + 1)],
                                             vals_fm[:, k, :],
                                             start=(k == 0), stop=(k == NK - 1))
                        t = tmp.tile([128, TOK], F32, tag="lnt")
                        nc.scalar.activation(t[:], ps[:], AF.Identity,
                                             bias=projb_a[:, l, m:m + 1])
                        nc.vector.tensor_add(x_fm[:, m, :], x_fm[:, m, :], t[:])

                # LN2 + FFN
                gs = [n2g_a[:, l, k:k + 1] for k in range(NK)]
                bs = [n2b_a[:, l, k:k + 1] for k in range(NK)]
                _ln(nc, tc, pools, x_fm, gs, bs, h_bf)

                with tc.tile_pool(name="wf1", bufs=NK) as pw:
                    wf = []
                    for k in range(NK):
                        w = pw.tile([128, HID], BF16, tag="w")
                        nc.sync.dma_start(w[:], d_f1w.ap()[l, k])
                        wf.append(w)
                    for m in range(NM_HID):
                        ps = ps_mm.tile([128, TOK], F32, tag="mm")
                        for k in range(NK):
                            nc.tensor.matmul(ps[:], wf[k][:, 128 * m:128 * (m + 1)],
                                             h_bf[:, k, :],
                                             start=(k == 0), stop=(k == NK - 1))
                        nc.scalar.activation(h2_sb[:, m, :], ps[:], AF.Gelu,
                                             bias=f1b_a[:, l, m:m + 1])

                with tc.tile_pool(name="wf2", bufs=NM_HID) as pw:
                    wf = []
                    for k in range(NM_HID):
                        w = pw.tile([128, D], BF16, tag="w")
                        nc.sync.dma_start(w[:], d_f2w.ap()[l, k])
                        wf.append(w)
                    for m in range(NK):
                        ps = ps_mm.tile([128, TOK], F32, tag="mm")
                        for k in range(NM_HID):
                            nc.tensor.matmul(ps[:], wf[k][:, 128 * m:128 * (m + 1)],
                                             h2_sb[:, k, :],
                                             start=(k == 0), stop=(k == NM_HID - 1))
                        t = tmp.tile([128, TOK], F32, tag="lnt")
                        nc.scalar.activation(t[:], ps[:], AF.Identity,
                                             bias=f2b_a[:, l, m:m + 1])
                        nc.vector.tensor_add(x_fm[:, m, :], x_fm[:, m, :], t[:])

            # final LN + lm_head
            gs = [fing_a[:, k:k + 1] for k in range(NK)]
            bs = [finb_a[:, k:k + 1] for k in range(NK)]
            _ln(nc, tc, pools, x_fm, gs, bs, h_bf)

            with tc.tile_pool(name="wlm", bufs=12) as pw:
                for b in range(NVB):
                    wvs = []
                    for k in range(NK):
                        w = pw.tile([128, VB], BF16, tag="w")
                        nc.sync.dma_start(w[:], d_wemb.ap()[b, k])
                        wvs.append(w)
                    for qt in range(2):
                        ps = ps_sc.tile([128, VB], F32, tag="sc")
                        for k in range(NK):
                            nc.tensor.matmul(ps[:],
                                             h_bf[:, k, 128 * qt:128 * (qt + 1)],
                                             wvs[k][:],
                                             start=(k == 0), stop=(k == NK - 1))
                        ot = lmo.tile([128, VB], F32, tag="ot")
                        nc.vector.tensor_copy(ot[:], ps[:])
                        nc.sync.dma_start(
                            d_out.ap()[128 * qt:128 * (qt + 1), VB * b:VB * (b + 1)],
                            ot[:])

    nc.compile()
    return nc


def _prep_inputs(W_emb, pos_emb, norm1_g, norm1_b, qkv_w, qkv_b, proj_w, proj_b,
                 norm2_g, norm2_b, ffn_w1, ffn_b1, ffn_w2, ffn_b2, fin_g, fin_b,
                 input_ids):
    bf = ml_dtypes.bfloat16
    f32 = np.float32

    def tp(a):  # [L, out, in] -> [L, NK, 128, out] bf16
        a = np.asarray(a, f32)
        out_dim = a.shape[1]
        return np.ascontiguousarray(
            a.transpose(0, 2, 1).reshape(L, NK, 128, out_dim)).astype(bf)

    def btile(a, nm):  # [L, nm*128] -> [L, 128, nm]
        return np.ascontiguousarray(
            np.asarray(a, f32).reshape(L, nm, 128).transpose(0, 2, 1))

    qkv_r = np.asarray(qkv_w, f32).reshape(L, H, 3, HD, D).transpose(0, 2, 1, 3, 4) \
        .reshape(L, 3 * D, D)
    qkv_b_r = np.asarray(qkv_b, f32).reshape(L, H, 3, HD).transpose(0, 2, 1, 3) \
        .reshape(L, 3 * D).copy()
    qkv_b_r[:, :D] *= 1.0 / math.sqrt(HD)   # q bias shares the score scale

    f2w = np.asarray(ffn_w2, f32)  # [L, D, HID]
    f2w_t = np.ascontiguousarray(
        f2w.transpose(0, 2, 1).reshape(L, NM_HID, 128, D)).astype(bf)

    W_emb = np.asarray(W_emb, f32)
    wemb_t = np.ascontiguousarray(
        W_emb.T.reshape(NK, 128, NVB, VB).transpose(2, 0, 1, 3)).astype(bf)

    ids = np.asarray(input_ids).reshape(-1).astype(np.int64)
    x0 = W_emb[ids] * math.sqrt(D)
    x0 = x0 + np.asarray(pos_emb, f32)[np.tile(np.arange(S), B)]

    common = {
        "qkvw": tp(qkv_r), "qkvb": btile(qkv_b_r, NM_QKV),
        "projw": tp(np.asarray(proj_w, f32)), "projb": btile(proj_b, NK),
        "f1w": tp(np.asarray(ffn_w1, f32)), "f1b": btile(ffn_b1, NM_HID),
        "f2w": f2w_t, "f2b": btile(ffn_b2, NK),
        "n1g": btile(norm1_g, NK), "n1b": btile(norm1_b, NK),
        "n2g": btile(norm2_g, NK), "n2b": btile(norm2_b, NK),
        "fing": np.ascontiguousarray(np.asarray(fin_g, f32).reshape(NK, 128).T),
        "finb": np.ascontiguousarray(np.asarray(fin_b, f32).reshape(NK, 128).T),
        "wemb": wemb_t,
    }

    kg = np.arange(4 * TOK)
    in_maps = []
    for c in range(N_CORES):
        xs = np.ascontiguousarray(
            x0[TOK * c:TOK * (c + 1)].T.reshape(NK, 128, TOK)).astype(f32)
        p = c % 4
        qg = p * TOK + np.arange(TOK)
        m = np.where(qg[None, :] >= kg[:, None], 0.0, -1e9).astype(f32)
        m = np.ascontiguousarray(m.reshape(8, 128, TOK))
        in_maps.append({"x0": xs, "mask": m, **common})
    return in_maps


def kernel(**inputs):
    global LAST_RESULT, _NC_CACHE
    in_maps = _prep_inputs(**inputs)
    if _NC_CACHE is None:
        _NC_CACHE = build_nc()
    res = run_bass_kernel_spmd(_NC_CACHE, in_maps, list(range(N_CORES)),
                               trace=TRACE)
    LAST_RESULT = res
    logits = np.concatenate(
        [np.asarray(res.results[c]["logits"]) for c in range(N_CORES)], axis=0)
    return logits.reshape(B, S, V).astype(np.float32)



# revision 10
# speedup vs baseline: 1.1879x; 1.1879x over previous
"""GPT decoder (V=32000,S=1024,D=768,H=12,HID=3072,L=4,B=2) on 8 trn2 cores.

Sharding: sequence-parallel body — core c owns tokens [256c, 256c+256) of the
flattened [2048] token stream (cores 0-3 = batch 0, cores 4-7 = batch 1).
Per layer K then V are exchanged with two pipelined AllGathers inside each
4-core batch group (overlapped with Q/V compute and scores); attention/FFN
stay local.  The tied lm_head runs per-core over the full vocab for the
local 256 tokens.  LayerNorm affine params are folded into the adjacent
weights host-side; softmax masking is multiplicative (0/1) after exp.
Matmuls in bf16 with fp32 PSUM accumulation; logits stored bf16.
Activations are feature-major [D, tok] so contractions sit on partitions.
"""
import math

import ml_dtypes
import numpy as np

import concourse.bass as bass
import concourse.mybir as mybir
import concourse.tile as tile
from concourse import bacc
from concourse.bass_utils import run_bass_kernel_spmd

F32 = mybir.dt.float32
BF16 = mybir.dt.bfloat16
AF = mybir.ActivationFunctionType
ALU = mybir.AluOpType

N_CORES = 8
GROUPS = [[0, 1, 2, 3], [4, 5, 6, 7]]
V, S, D, H, HID, L, B = 32000, 1024, 768, 12, 3072, 4, 2
HD = D // H          # 64
TOK = 256            # tokens per core
NK = D // 128        # 6 feature chunks
NM_QKV = 3 * D // 128   # 18
NM_HID = HID // 128     # 24
EPS = 1e-5
VB = 500             # lm_head vocab block
NVB = V // VB        # 64
NBIAS = NM_QKV + NK + NM_HID + NK   # 54 bias cols per layer

TRACE = False
LAST_RESULT = None

_NC_CACHE = None


def _ln(nc, pools, x_fm, h_bf):
    """h_bf = (x - mean)/sqrt(var+eps), feature dim on partitions.

    Stats via ones-matmul partition reductions; rstd computed on a
    128-partition broadcast so DVE reciprocal runs wide (not 1-lane).
    """
    ps_stat, ps_mm, tmp = pools["ps_stat"], pools["ps_mm"], pools["tmp"]
    ones_bf, invD_row = pools["ones_bf"], pools["invD_row"]

    s1 = ps_stat.tile([1, TOK], F32, tag="stat")
    s2 = ps_stat.tile([1, TOK], F32, tag="stat")
    for k in range(NK):
        xb = tmp.tile([128, TOK], BF16, tag="lnxb")
        nc.vector.tensor_copy(xb[:], x_fm[:, k, :])
        nc.tensor.matmul(s1[:], ones_bf[:], xb[:], start=(k == 0), stop=(k == NK - 1))
        sq = tmp.tile([128, TOK], BF16, tag="lnsq")
        nc.vector.tensor_mul(sq[:], xb[:], xb[:])
        nc.tensor.matmul(s2[:], ones_bf[:], sq[:], start=(k == 0), stop=(k == NK - 1))

    s12 = tmp.tile([1, 2, TOK], BF16, tag="s12")
    nc.vector.tensor_copy(s12[:, 0, :], s1[:])
    nc.vector.tensor_copy(s12[:, 1, :], s2[:])
    mean_bc = ps_mm.tile([128, TOK], F32, tag="mm")
    nc.tensor.matmul(mean_bc[:], invD_row[:], s12[:, 0, :], start=True, stop=True)
    m2_bc = ps_mm.tile([128, TOK], F32, tag="mm")
    nc.tensor.matmul(m2_bc[:], invD_row[:], s12[:, 1, :], start=True, stop=True)

    msq = tmp.tile([128, TOK], F32, tag="lnf")
    nc.scalar.activation(msq[:], mean_bc[:], AF.Square)
    var = tmp.tile([128, TOK], F32, tag="lnf")
    nc.vector.tensor_sub(var[:], m2_bc[:], msq[:])
    sd = tmp.tile([128, TOK], F32, tag="lnf")
    nc.scalar.activation(sd[:], var[:], AF.Sqrt, bias=pools["eps_col"])
    rstd = tmp.tile([128, TOK], F32, tag="lnf")
    nc.vector.reciprocal(rstd[:], sd[:])

    for k in range(NK):
        t = tmp.tile([128, TOK], F32, tag="lnt")
        nc.vector.tensor_sub(t[:], x_fm[:, k, :], mean_bc[:])
        nc.vector.tensor_mul(h_bf[:, k, :], t[:], rstd[:])


def build_nc():
    nc = bacc.Bacc("TRN2", target_bir_lowering=False, debug=False,
                   enable_asserts=True, num_devices=N_CORES)

    d_x0 = nc.dram_tensor("x0", [128, NK, TOK], F32, kind="ExternalInput")
    d_mask = nc.dram_tensor("mask", [128, 8, TOK], BF16, kind="ExternalInput")
    d_bias = nc.dram_tensor("biases", [128, L, NBIAS], F32, kind="ExternalInput")
    d_qkvw = nc.dram_tensor("qkvw", [L, NK, 128, 3 * D], BF16, kind="ExternalInput")
    d_projw = nc.dram_tensor("projw", [L, NK, 128, D], BF16, kind="ExternalInput")
    d_f1w = nc.dram_tensor("f1w", [L, NK, 128, HID], BF16, kind="ExternalInput")
    d_f2w = nc.dram_tensor("f2w", [L, NM_HID, 128, D], BF16, kind="ExternalInput")
    d_wemb = nc.dram_tensor("wemb", [NVB, 128, NK, VB], BF16, kind="ExternalInput")
    d_out = nc.dram_tensor("logits", [128, 2, NVB, VB], BF16, kind="ExternalOutput")

    with tile.TileContext(nc) as tc:
        from contextlib import ExitStack
        with ExitStack() as ctx:
            const = ctx.enter_context(tc.tile_pool(name="const", bufs=1))
            res = ctx.enter_context(tc.tile_pool(name="res", bufs=1))
            tmp = ctx.enter_context(tc.tile_pool(name="tmp", bufs=3))
            attn = ctx.enter_context(tc.tile_pool(name="attn", bufs=3))
            lmo = ctx.enter_context(tc.tile_pool(name="lmo", bufs=4))
            ps_mm = ctx.enter_context(tc.tile_pool(name="ps_mm", bufs=2, space="PSUM"))
            ps_q = ctx.enter_context(tc.tile_pool(name="ps_q", bufs=2, space="PSUM"))
            ps_stat = ctx.enter_context(tc.tile_pool(name="ps_stat", bufs=2, space="PSUM"))
            dram = ctx.enter_context(tc.tile_pool(name="dram", bufs=2, space="DRAM"))

            identity = const.tile([128, 128], BF16)
            from concourse.masks import make_identity
            make_identity(nc, identity[:])
            ones_bf = const.tile([128, 1], BF16)
            nc.any.memset(ones_bf[:], 1.0)
            invD_row = const.tile([1, 128], BF16)
            nc.any.memset(invD_row[:], 1.0 / D)
            ones2 = const.tile([1, 64], F32)
            nc.any.memset(ones2[:], 1.0)
            eps_col = const.tile([128, 1], F32)
            nc.any.memset(eps_col[:], EPS)

            # Residual stream + mask + biases, resident in SBUF
            x_fm = res.tile([128, NK, TOK], F32)
            nc.sync.dma_start(x_fm[:], d_x0.ap())
            mask_t = res.tile([128, 8, TOK], BF16)
            nc.sync.dma_start(mask_t[:], d_mask.ap())
            bias_a = res.tile([128, L, NBIAS], F32)
            nc.sync.dma_start(bias_a[:], d_bias.ap())

            pools = dict(ps_stat=ps_stat, ps_mm=ps_mm, tmp=tmp,
                         ones_bf=ones_bf, invD_row=invD_row,
                         eps_col=eps_col[:])

            h_bf = res.tile([128, NK, TOK], BF16)
            qkv_sb = res.tile([128, NM_QKV, TOK], BF16)
            # v_own: per head 66 cols = [ones | v(64) | ones]; kept for the
            # same packed exchange layout as k (264-col slabs of 4 heads).
            v_own = res.tile([128, 2, H, 66], BF16)
            nc.any.memset(v_own[:, :, :, 0:1], 1.0)
            nc.any.memset(v_own[:, :, :, 65:66], 1.0)
            k_all = res.tile([128, 4, NK, TOK], BF16)      # [p, rank, kchunk, t]
            v_all = res.tile([128, 4, 2, H, 66], BF16)     # [p, rank, thalf, head, 66]
            vals_fm = res.tile([128, NK, TOK], BF16)
            h2_sb = res.tile([128, NM_HID, TOK], BF16)

            def mm_block(ps, wtiles, m, rhs_t, nk):
                """Accumulate ps += sum_k w[k][:,128m:128(m+1)].T @ rhs[:,k,:]."""
                for k in range(nk):
                    ti, j = wtiles[k]
                    nc.tensor.matmul(ps[:], ti[:, j, 128 * m:128 * (m + 1)],
                                     rhs_t[:, k, :], start=(k == 0), stop=(k == nk - 1))

            for l in range(L):
                _ln(nc, pools, x_fm, h_bf)

                with tc.tile_pool(name="wqkv", bufs=3) as pw:
                    wk = []
                    for j in range(3):
                        w = pw.tile([128, 2, 3 * D], BF16, tag="w")
                        nc.sync.dma_start(
                            w[:],
                            d_qkvw.ap()[l, 2 * j:2 * j + 2].rearrange("k p m -> p k m"))
                        wk.append(w)
                    wt = [(wk[k // 2], k % 2) for k in range(NK)]

                    # K chunks first so the gather can launch early
                    for m in range(NK, 2 * NK):
                        ps = ps_mm.tile([128, TOK], F32, tag="mm")
                        mm_block(ps, wt, m, h_bf, NK)
                        nc.scalar.activation(qkv_sb[:, m, :], ps[:], AF.Identity,
                                             bias=bias_a[:, l, m:m + 1])
                    bK_in = dram.tile([128, NK, TOK], BF16, tag="bkin")
                    bK_out = dram.tile([4, 128, NK, TOK], BF16, tag="bkout")
                    nc.sync.dma_start(bK_in[:], qkv_sb[:, NK:2 * NK, :])
                    nc.gpsimd.collective_compute(
                        "AllGather", ALU.bypass, replica_groups=GROUPS,
                        ins=[bK_in.opt()], outs=[bK_out.opt()])

                    # V chunks + transpose to token-major
                    for m in range(2 * NK, 3 * NK):
                        ps = ps_mm.tile([128, TOK], F32, tag="mm")
                        mm_block(ps, wt, m, h_bf, NK)
                        nc.scalar.activation(qkv_sb[:, m, :], ps[:], AF.Identity,
                                             bias=bias_a[:, l, m:m + 1])
                    for h in range(H):
                        o = (h % 2) * 64
                        for t in range(2):
                            src = qkv_sb[o:o + 64, 12 + h // 2, 128 * t:128 * (t + 1)]
                            pt = ps_q.tile([128, 64], BF16, tag="sc")
                            nc.tensor.transpose(pt[:], src,
                                                identity[o:o + 64, o:o + 64])
                            nc.vector.tensor_copy(v_own[:, t, h, 1:65], pt[:])
                    bV_in = dram.tile([128, 2, H, 66], BF16, tag="bvin")
                    bV_out = dram.tile([4, 128, 2, H, 66], BF16, tag="bvout")
                    nc.sync.dma_start(bV_in[:], v_own[:])
                    nc.gpsimd.collective_compute(
                        "AllGather", ALU.bypass, replica_groups=GROUPS,
                        ins=[bV_in.opt()], outs=[bV_out.opt()])

                    # Q chunks (overlap the K AllGather)
                    for m in range(NK):
                        ps = ps_mm.tile([128, TOK], F32, tag="mm")
                        mm_block(ps, wt, m, h_bf, NK)
                        nc.scalar.activation(qkv_sb[:, m, :], ps[:], AF.Identity,
                                             bias=bias_a[:, l, m:m + 1])

                nc.sync.dma_start(k_all[:], bK_out[:].rearrange("c p k t -> p c k t"))
                nc.sync.dma_start(v_all[:], bV_out[:].rearrange("c p t h x -> p c t h x"))

                # attention: scores per 4-chunk quad -> exp -> mask-mul -> AV
                for h in range(H):
                    o = (h % 2) * 64
                    kslc = slice(o, o + 64)
                    av = ps_mm.tile([128, TOK], F32, tag="mm")
                    dn = ps_stat.tile([1, TOK], F32, tag="stat")
                    for half in range(2):
                        st = ps_q.tile([128, 4, TOK], F32, tag="sc")
                        for j in range(4):
                            kc = 4 * half + j
                            c, hf = kc // 2, kc % 2
                            nc.tensor.matmul(
                                st[:, j, :],
                                k_all[kslc, c, h // 2, 128 * hf:128 * (hf + 1)],
                                qkv_sb[kslc, h // 2, :],
                                start=True, stop=True)
                        pt_t = attn.tile([128, 4, TOK], BF16, tag="ptsb")
                        nc.scalar.activation(pt_t[:], st[:], AF.Exp)
                        nc.vector.tensor_mul(pt_t[:], pt_t[:],
                                             mask_t[:, 4 * half:4 * half + 4, :])
                        for j in range(4):
                            kc = 4 * half + j
                            c, hf = kc // 2, kc % 2
                            nc.tensor.matmul(av[o:o + 64, :],
                                             v_all[:, c, hf, h, 1:65],
                                             pt_t[:, j, :],
                                             start=(kc == 0), stop=(kc == 7))
                            nc.tensor.matmul(dn[:], ones_bf[:], pt_t[:, j, :],
                                             start=(kc == 0), stop=(kc == 7))
                    dn_sb = tmp.tile([1, TOK], F32, tag="dnsb")
                    nc.vector.tensor_copy(dn_sb[:], dn[:])
                    bc = ps_q.tile([128, TOK], F32, tag="sc")
                    nc.tensor.matmul(bc[o:o + 64, :], ones2[:],
                                     dn_sb[:], start=True, stop=True)
                    rv = tmp.tile([128, TOK], F32, tag="rv")
                    nc.vector.reciprocal(rv[o:o + 64, :], bc[o:o + 64, :])
                    nc.vector.tensor_mul(vals_fm[o:o + 64, h // 2, :],
                                         av[o:o + 64, :], rv[o:o + 64, :])

                # proj + residual
                with tc.tile_pool(name="wproj", bufs=2) as pw:
                    pk = []
                    for j in range(2):
                        w = pw.tile([128, 3, D], BF16, tag="w")
                        nc.sync.dma_start(
                            w[:],
                            d_projw.ap()[l, 3 * j:3 * j + 3].rearrange("k p m -> p k m"))
                        pk.append(w)
                    wt = [(pk[k // 3], k % 3) for k in range(NK)]
                    for m in range(NK):
                        ps = ps_mm.tile([128, TOK], F32, tag="mm")
                        mm_block(ps, wt, m, vals_fm, NK)
                        t = tmp.tile([128, TOK], F32, tag="lnt")
                        nc.scalar.activation(t[:], ps[:], AF.Identity,
                                             bias=bias_a[:, l, NM_QKV + m:NM_QKV + m + 1])
                        nc.vector.tensor_add(x_fm[:, m, :], x_fm[:, m, :], t[:])

                _ln(nc, pools, x_fm, h_bf)

                with tc.tile_pool(name="wf1", bufs=3) as pw:
                    wf = []
                    for j in range(3):
                        w = pw.tile([128, 2, HID], BF16, tag="w")
                        nc.sync.dma_start(
                            w[:],
                            d_f1w.ap()[l, 2 * j:2 * j + 2].rearrange("k p m -> p k m"))
                        wf.append(w)
                    wt = [(wf[k // 2], k % 2) for k in range(NK)]
                    for m in range(NM_HID):
                        ps = ps_mm.tile([128, TOK], F32, tag="mm")
                        mm_block(ps, wt, m, h_bf, NK)
                        nc.scalar.activation(h2_sb[:, m, :], ps[:], AF.Gelu,
                                             bias=bias_a[:, l, 24 + m:24 + m + 1])

                with tc.tile_pool(name="wf2", bufs=4) as pw:
                    wf = []
                    for j in range(4):
                        w = pw.tile([128, 6, D], BF16, tag="w")
                        nc.sync.dma_start(
                            w[:],
                            d_f2w.ap()[l, 6 * j:6 * j + 6].rearrange("k p m -> p k m"))
                        wf.append(w)
                    wt = [(wf[k // 6], k % 6) for k in range(NM_HID)]
                    for m in range(NK):
                        ps = ps_mm.tile([128, TOK], F32, tag="mm")
                        mm_block(ps, wt, m, h2_sb, NM_HID)
                        t = tmp.tile([128, TOK], F32, tag="lnt")
                        nc.scalar.activation(t[:], ps[:], AF.Identity,
                                             bias=bias_a[:, l, 48 + m:48 + m + 1])
                        nc.vector.tensor_add(x_fm[:, m, :], x_fm[:, m, :], t[:])

            # final LN + lm_head
            _ln(nc, pools, x_fm, h_bf)

            with tc.tile_pool(name="wlm", bufs=4) as pw:
                for b in range(NVB):
                    w = pw.tile([128, NK, VB], BF16, tag="w")
                    nc.sync.dma_start(w[:], d_wemb.ap()[b])
                    ot = lmo.tile([128, 2, VB], BF16, tag="ot")
                    for qt in range(2):
                        ps = ps_q.tile([128, VB], F32, tag="sc")
                        for k in range(NK):
                            nc.tensor.matmul(ps[:],
                                             h_bf[:, k, 128 * qt:128 * (qt + 1)],
                                             w[:, k, :],
                                             start=(k == 0), stop=(k == NK - 1))
                        nc.vector.tensor_copy(ot[:, qt, :], ps[:])
                    nc.sync.dma_start(d_out.ap()[:, :, b, :], ot[:])

    nc.compile()
    return nc


def _prep_inputs(W_emb, pos_emb, norm1_g, norm1_b, qkv_w, qkv_b, proj_w, proj_b,
                 norm2_g, norm2_b, ffn_w1, ffn_b1, ffn_w2, ffn_b2, fin_g, fin_b,
                 input_ids):
    bf = ml_dtypes.bfloat16
    f32 = np.float32

    W_emb = np.asarray(W_emb, f32)
    pos_emb = np.asarray(pos_emb, f32)
    qkv_w = np.asarray(qkv_w, f32)
    qkv_b = np.asarray(qkv_b, f32)
    proj_w = np.asarray(proj_w, f32)
    proj_b = np.asarray(proj_b, f32)
    ffn_w1 = np.asarray(ffn_w1, f32)
    ffn_b1 = np.asarray(ffn_b1, f32)
    ffn_w2 = np.asarray(ffn_w2, f32)
    ffn_b2 = np.asarray(ffn_b2, f32)
    n1g, n1b = np.asarray(norm1_g, f32), np.asarray(norm1_b, f32)
    n2g, n2b = np.asarray(norm2_g, f32), np.asarray(norm2_b, f32)
    fin_g, fin_b = np.asarray(fin_g, f32), np.asarray(fin_b, f32)

    # Fold LN affines into the consuming weights
    qkv_w_eff = qkv_w * n1g[:, None, :]                       # [L,3D,D]
    qkv_b_eff = qkv_b + np.einsum("lod,ld->lo", qkv_w, n1b)
    f1w_eff = ffn_w1 * n2g[:, None, :]
    f1b_eff = ffn_b1 + np.einsum("lod,ld->lo", ffn_w1, n2b)
    wemb_eff = W_emb * fin_g[None, :]                         # lm_head side only
    lm_bias = W_emb @ fin_b                                   # [V], host-added

    # head-permute qkv to [q(all heads) | k | v], fold 1/sqrt(HD) into q
    qkv_r = qkv_w_eff.reshape(L, H, 3, HD, D).transpose(0, 2, 1, 3, 4) \
        .reshape(L, 3 * D, D).copy()
    qkv_b_r = qkv_b_eff.reshape(L, H, 3, HD).transpose(0, 2, 1, 3) \
        .reshape(L, 3 * D).copy()
    sc = 1.0 / math.sqrt(HD)
    qkv_r[:, :D, :] *= sc
    qkv_b_r[:, :D] *= sc

    def tp(a):  # [L, out, in] -> [L, NK, 128, out] bf16
        out_dim = a.shape[1]
        return np.ascontiguousarray(
            a.transpose(0, 2, 1).reshape(L, NK, 128, out_dim)).astype(bf)

    def btile(a, nm):  # [L, nm*128] -> [L, 128, nm]
        return np.ascontiguousarray(a.reshape(L, nm, 128).transpose(0, 2, 1))

    f2w_t = np.ascontiguousarray(
        ffn_w2.transpose(0, 2, 1).reshape(L, NM_HID, 128, D)).astype(bf)

    biases = np.concatenate([
        btile(qkv_b_r, NM_QKV), btile(proj_b, NK),
        btile(f1b_eff, NM_HID), btile(ffn_b2, NK)], axis=2)   # [L,128,54]
    biases = np.ascontiguousarray(biases.transpose(1, 0, 2))  # [128,L,54]

    wemb_t = np.ascontiguousarray(
        wemb_eff.T.reshape(NK, 128, NVB, VB).transpose(2, 1, 0, 3)).astype(bf)

    ids = np.asarray(input_ids).reshape(-1).astype(np.int64)
    x0 = W_emb[ids] * math.sqrt(D)
    x0 = x0 + pos_emb[np.tile(np.arange(S), B)]

    common = {
        "qkvw": tp(qkv_r),
        "projw": tp(proj_w),
        "f1w": tp(f1w_eff),
        "f2w": f2w_t,
        "biases": biases,
        "wemb": wemb_t,
    }

    kg = np.arange(4 * TOK)
    in_maps = []
    for c in range(N_CORES):
        # [128, NK, TOK]: element [p, k, t] = x0[t, k*128+p]
        xs = np.ascontiguousarray(
            x0[TOK * c:TOK * (c + 1)].T.reshape(NK, 128, TOK).transpose(1, 0, 2))
        p = c % 4
        qg = p * TOK + np.arange(TOK)
        m = np.where(qg[None, :] >= kg[:, None], 1.0, 0.0)
        m = np.ascontiguousarray(m.reshape(8, 128, TOK).transpose(1, 0, 2)).astype(bf)
        in_maps.append({"x0": xs.astype(f32), "mask": m, **common})
    return in_maps, lm_bias


def kernel(**inputs):
    global LAST_RESULT, _NC_CACHE
    in_maps, lm_bias = _prep_inputs(**inputs)
    if _NC_CACHE is None:
        _NC_CACHE = build_nc()
    res = run_bass_kernel_spmd(_NC_CACHE, in_maps, list(range(N_CORES)),
                               trace=TRACE)
    LAST_RESULT = res
    outs = []
    for c in range(N_CORES):
        o = np.asarray(res.results[c]["logits"]).astype(np.float32)
        # [128, 2, NVB, VB] -> [256, V]
        outs.append(o.transpose(1, 0, 2, 3).reshape(TOK, V))
    logits = np.concatenate(outs, axis=0).reshape(B, S, V)
    return (logits + lm_bias[None, None, :]).astype(np.float32)


# revision 13
# speedup vs baseline: 1.4309x; 1.2045x over previous
"""GPT decoder (V=32000,S=1024,D=768,H=12,HID=3072,L=4,B=2) on 8 trn2 cores.

Sharding: sequence-parallel body — core c owns tokens [256c, 256c+256) of the
flattened [2048] token stream (cores 0-3 = batch 0, cores 4-7 = batch 1).
Per layer K (split in two) then V are exchanged with pipelined AllGathers
inside each 4-core batch group; score matmuls (K-only) run while the V
exchange flies, AV runs after, so the in-order PE queue never blocks on a
collective.  The tied lm_head runs per-core over the full vocab for the
local 256 tokens.  LayerNorm affine params are folded into the adjacent
weights host-side; softmax masking is multiplicative (0/1) after exp.
Matmuls in bf16 with fp32 PSUM accumulation; logits stored bf16.
Activations are feature-major [D, tok] so contractions sit on partitions.
"""
import math

import ml_dtypes
import numpy as np

import concourse.bass as bass
import concourse.mybir as mybir
import concourse.tile as tile
from concourse import bacc
from concourse.bass_utils import run_bass_kernel_spmd

F32 = mybir.dt.float32
BF16 = mybir.dt.bfloat16
AF = mybir.ActivationFunctionType
ALU = mybir.AluOpType

N_CORES = 8
GROUPS = [[0, 1, 2, 3], [4, 5, 6, 7]]
V, S, D, H, HID, L, B = 32000, 1024, 768, 12, 3072, 4, 2
HD = D // H          # 64
TOK = 256            # tokens per core
NK = D // 128        # 6 feature chunks
NM_QKV = 3 * D // 128   # 18
NM_HID = HID // 128     # 24
EPS = 1e-5
VB = 500             # lm_head vocab block
NVB = V // VB        # 64
NBIAS = NM_QKV + NK + NM_HID + NK   # 54 bias cols per layer

TRACE = False
LAST_RESULT = None

_NC_CACHE = None


def build_nc():
    nc = bacc.Bacc("TRN2", target_bir_lowering=False, debug=False,
                   enable_asserts=True, num_devices=N_CORES)

    d_x0 = nc.dram_tensor("x0", [128, NK, TOK], F32, kind="ExternalInput")
    d_mask = nc.dram_tensor("mask", [128, 8, TOK], BF16, kind="ExternalInput")
    d_bias = nc.dram_tensor("biases", [128, L, NBIAS], F32, kind="ExternalInput")
    d_qkvw = nc.dram_tensor("qkvw", [L, NK, 128, 3 * D], BF16, kind="ExternalInput")
    d_projw = nc.dram_tensor("projw", [L, NK, 128, D], BF16, kind="ExternalInput")
    d_f1w = nc.dram_tensor("f1w", [L, NK, 128, HID], BF16, kind="ExternalInput")
    d_f2w = nc.dram_tensor("f2w", [L, NM_HID, 128, D], BF16, kind="ExternalInput")
    d_wemb = nc.dram_tensor("wemb", [NVB, 128, NK, VB], BF16, kind="ExternalInput")
    d_out = nc.dram_tensor("logits", [128, 2, NVB, VB], BF16, kind="ExternalOutput")

    with tile.TileContext(nc) as tc:
        from contextlib import ExitStack
        with ExitStack() as ctx:
            const = ctx.enter_context(tc.tile_pool(name="const", bufs=1))
            res = ctx.enter_context(tc.tile_pool(name="res", bufs=1))
            tmp = ctx.enter_context(tc.tile_pool(name="tmp", bufs=3))
            lmo = ctx.enter_context(tc.tile_pool(name="lmo", bufs=6))
            ps_mm = ctx.enter_context(tc.tile_pool(name="ps_mm", bufs=2, space="PSUM"))
            ps_q = ctx.enter_context(tc.tile_pool(name="ps_q", bufs=2, space="PSUM"))
            ps_stat = ctx.enter_context(tc.tile_pool(name="ps_stat", bufs=2, space="PSUM"))
            dram = ctx.enter_context(tc.tile_pool(name="dram", bufs=2, space="DRAM"))

            identity = const.tile([128, 128], BF16)
            from concourse.masks import make_identity
            make_identity(nc, identity[:])
            ones_bf = const.tile([128, 1], BF16)
            nc.any.memset(ones_bf[:], 1.0)
            invD_row = const.tile([1, 128], BF16)
            nc.any.memset(invD_row[:], 1.0 / D)
            ones2 = const.tile([1, 64], F32)
            nc.any.memset(ones2[:], 1.0)
            eps_col = const.tile([128, 1], F32)
            nc.any.memset(eps_col[:], EPS)

            # Residual stream + mask + biases, resident in SBUF
            x_fm = res.tile([128, NK, TOK], F32)
            nc.sync.dma_start(x_fm[:], d_x0.ap())
            mask_t = res.tile([128, 8, TOK], BF16)
            nc.sync.dma_start(mask_t[:], d_mask.ap())
            bias_a = res.tile([128, L, NBIAS], F32)
            nc.sync.dma_start(bias_a[:], d_bias.ap())

            h_bf = res.tile([128, NK, TOK], BF16)
            xsq = res.tile([128, 2, NK, TOK], BF16)   # LN stats scratch
            qkv_sb = res.tile([128, NM_QKV, TOK], BF16)
            # v_own: per head 66 cols = [ones | v(64) | ones] (264-col slabs)
            v_own = res.tile([128, 2, H, 66], BF16)
            nc.any.memset(v_own[:, :, :, 0:1], 1.0)
            nc.any.memset(v_own[:, :, :, 65:66], 1.0)
            k_lo = res.tile([128, 4, 3, TOK], BF16)   # [p, rank, kchunk 0-2, t]
            k_hi = res.tile([128, 4, 3, TOK], BF16)   # [p, rank, kchunk 3-5, t]
            v_all = res.tile([128, 4, 2, H, 66], BF16)
            pt_all = res.tile([128, H, 8, TOK], BF16)  # exp'd masked scores
            vals_fm = res.tile([128, NK, TOK], BF16)
            h2_sb = res.tile([128, NM_HID, TOK], BF16)

            def _ln(out_bf):
                """out_bf = (x - mean)/sqrt(var+eps) over features."""
                for k in range(NK):
                    nc.vector.tensor_copy(xsq[:, 0, k, :], x_fm[:, k, :])
                    nc.vector.tensor_mul(xsq[:, 1, k, :], xsq[:, 0, k, :],
                                         xsq[:, 0, k, :])
                s1 = ps_stat.tile([1, TOK], F32, tag="stat")
                s2 = ps_stat.tile([1, TOK], F32, tag="stat")
                for k in range(NK):
                    nc.tensor.matmul(s1[:], ones_bf[:], xsq[:, 0, k, :],
                                     start=(k == 0), stop=(k == NK - 1))
                    nc.tensor.matmul(s2[:], ones_bf[:], xsq[:, 1, k, :],
                                     start=(k == 0), stop=(k == NK - 1))
                s12 = tmp.tile([1, 2, TOK], BF16, tag="s12")
                nc.vector.tensor_copy(s12[:, 0, :], s1[:])
                nc.vector.tensor_copy(s12[:, 1, :], s2[:])
                mean_bc = ps_mm.tile([128, TOK], F32, tag="mm")
                nc.tensor.matmul(mean_bc[:], invD_row[:], s12[:, 0, :],
                                 start=True, stop=True)
                m2_bc = ps_mm.tile([128, TOK], F32, tag="mm")
                nc.tensor.matmul(m2_bc[:], invD_row[:], s12[:, 1, :],
                                 start=True, stop=True)
                msq = tmp.tile([128, TOK], F32, tag="lnf")
                nc.scalar.activation(msq[:], mean_bc[:], AF.Square)
                var = tmp.tile([128, TOK], F32, tag="lnf")
                nc.vector.tensor_sub(var[:], m2_bc[:], msq[:])
                sd = tmp.tile([128, TOK], F32, tag="lnf")
                nc.scalar.activation(sd[:], var[:], AF.Sqrt, bias=eps_col[:])
                rstd = tmp.tile([128, TOK], F32, tag="lnf")
                nc.vector.reciprocal(rstd[:], sd[:])
                for k in range(NK):
                    t = tmp.tile([128, TOK], F32, tag="lnt")
                    nc.vector.tensor_sub(t[:], x_fm[:, k, :], mean_bc[:])
                    nc.vector.tensor_mul(out_bf[:, k, :], t[:], rstd[:])

            def gemm_ps(i):
                """Alternate PSUM pools so 4 accumulation groups are in flight."""
                if i % 2 == 0:
                    return ps_mm.tile([128, TOK], F32, tag="mm", name="gps")
                return ps_q.tile([128, TOK], F32, tag="sc", name="gps")

            def mm_block(ps, wtiles, m, rhs_t, nk):
                for k in range(nk):
                    ti, j = wtiles[k]
                    nc.tensor.matmul(ps[:], ti[:, j, 128 * m:128 * (m + 1)],
                                     rhs_t[:, k, :], start=(k == 0), stop=(k == nk - 1))

            for l in range(L):
                _ln(h_bf)

                with tc.tile_pool(name="wqkv", bufs=3) as pw:
                    wk = []
                    for j in range(3):
                        w = pw.tile([128, 2, 3 * D], BF16, tag="w")
                        nc.sync.dma_start(
                            w[:],
                            d_qkvw.ap()[l, 2 * j:2 * j + 2].rearrange("k p m -> p k m"))
                        wk.append(w)
                    wt = [(wk[k // 2], k % 2) for k in range(NK)]

                    # K chunks first, in two halves, so gathers launch early
                    for i, m in enumerate(range(NK, 2 * NK)):
                        ps = gemm_ps(i)
                        mm_block(ps, wt, m, h_bf, NK)
                        nc.scalar.activation(qkv_sb[:, m, :], ps[:], AF.Identity,
                                             bias=bias_a[:, l, m:m + 1])
                        if m == NK + 2:
                            bK1_in = dram.tile([128, 3, TOK], BF16, tag="bk1i")
                            bK1_out = dram.tile([4, 128, 3, TOK], BF16, tag="bk1o")
                            nc.sync.dma_start(bK1_in[:], qkv_sb[:, NK:NK + 3, :])
                            nc.gpsimd.collective_compute(
                                "AllGather", ALU.bypass, replica_groups=GROUPS,
                                ins=[bK1_in.opt()], outs=[bK1_out.opt()])
                    bK2_in = dram.tile([128, 3, TOK], BF16, tag="bk2i")
                    bK2_out = dram.tile([4, 128, 3, TOK], BF16, tag="bk2o")
                    nc.sync.dma_start(bK2_in[:], qkv_sb[:, NK + 3:2 * NK, :])
                    nc.gpsimd.collective_compute(
                        "AllGather", ALU.bypass, replica_groups=GROUPS,
                        ins=[bK2_in.opt()], outs=[bK2_out.opt()])

                    # V chunks + transpose to token-major
                    for i, m in enumerate(range(2 * NK, 3 * NK)):
                        ps = gemm_ps(i)
                        mm_block(ps, wt, m, h_bf, NK)
                        nc.scalar.activation(qkv_sb[:, m, :], ps[:], AF.Identity,
                                             bias=bias_a[:, l, m:m + 1])
                    for h in range(H):
                        o = (h % 2) * 64
                        for t in range(2):
                            src = qkv_sb[o:o + 64, 12 + h // 2, 128 * t:128 * (t + 1)]
                            pt = ps_q.tile([128, 64], BF16, tag="sc")
                            nc.tensor.transpose(pt[:], src,
                                                identity[o:o + 64, o:o + 64])
                            nc.vector.tensor_copy(v_own[:, t, h, 1:65], pt[:])
                    bV_in = dram.tile([128, 2, H, 66], BF16, tag="bvin")
                    bV_out = dram.tile([4, 128, 2, H, 66], BF16, tag="bvout")
                    nc.sync.dma_start(bV_in[:], v_own[:])
                    nc.gpsimd.collective_compute(
                        "AllGather", ALU.bypass, replica_groups=GROUPS,
                        ins=[bV_in.opt()], outs=[bV_out.opt()])

                    # Q chunks (overlap the K gathers)
                    for i, m in enumerate(range(NK)):
                        ps = gemm_ps(i)
                        mm_block(ps, wt, m, h_bf, NK)
                        nc.scalar.activation(qkv_sb[:, m, :], ps[:], AF.Identity,
                                             bias=bias_a[:, l, m:m + 1])

                nc.sync.dma_start(k_lo[:], bK1_out[:].rearrange("c p k t -> p c k t"))
                nc.sync.dma_start(k_hi[:], bK2_out[:].rearrange("c p k t -> p c k t"))
                nc.sync.dma_start(v_all[:], bV_out[:].rearrange("c p t h x -> p c t h x"))

                # scores (K only) -> exp -> mask, all heads, before any AV
                dns = {}
                for h in range(H):
                    o = (h % 2) * 64
                    kslc = slice(o, o + 64)
                    kt = h // 2
                    ksrc = k_lo if kt < 3 else k_hi
                    kj = kt % 3
                    for half in range(2):
                        st = ps_q.tile([128, 4, TOK], F32, tag="sc")
                        for j in range(4):
                            kc = 4 * half + j
                            c, hf = kc // 2, kc % 2
                            nc.tensor.matmul(
                                st[:, j, :],
                                ksrc[kslc, c, kj, 128 * hf:128 * (hf + 1)],
                                qkv_sb[kslc, h // 2, :],
                                start=True, stop=True)
                        sl = pt_all[:, h, 4 * half:4 * half + 4, :]
                        nc.scalar.activation(sl, st[:], AF.Exp)
                        nc.vector.tensor_mul(sl, sl,
                                             mask_t[:, 4 * half:4 * half + 4, :])
                    dn = dns[h] = ps_stat.tile([1, TOK], F32, tag="stat", name="dn")
                    for kc in range(8):
                        nc.tensor.matmul(dn[:], ones_bf[:], pt_all[:, h, kc, :],
                                         start=(kc == 0), stop=(kc == 7))

                # AV + normalize
                for h in range(H):
                    o = (h % 2) * 64
                    av = ps_mm.tile([128, TOK], F32, tag="mm")
                    dn = dns[h]
                    for kc in range(8):
                        c, hf = kc // 2, kc % 2
                        nc.tensor.matmul(av[o:o + 64, :],
                                         v_all[:, c, hf, h, 1:65],
                                         pt_all[:, h, kc, :],
                                         start=(kc == 0), stop=(kc == 7))
                    dn_sb = tmp.tile([1, TOK], F32, tag="dnsb")
                    nc.vector.tensor_copy(dn_sb[:], dn[:])
                    bc = ps_q.tile([128, TOK], F32, tag="sc")
                    nc.tensor.matmul(bc[o:o + 64, :], ones2[:],
                                     dn_sb[:], start=True, stop=True)
                    rv = tmp.tile([128, TOK], F32, tag="rv")
                    nc.vector.reciprocal(rv[o:o + 64, :], bc[o:o + 64, :])
                    nc.vector.tensor_mul(vals_fm[o:o + 64, h // 2, :],
                                         av[o:o + 64, :], rv[o:o + 64, :])

                # proj + residual
                with tc.tile_pool(name="wproj", bufs=2) as pw:
                    pk = []
                    for j in range(2):
                        w = pw.tile([128, 3, D], BF16, tag="w")
                        nc.sync.dma_start(
                            w[:],
                            d_projw.ap()[l, 3 * j:3 * j + 3].rearrange("k p m -> p k m"))
                        pk.append(w)
                    wt = [(pk[k // 3], k % 3) for k in range(NK)]
                    for m in range(NK):
                        ps = gemm_ps(m)
                        mm_block(ps, wt, m, vals_fm, NK)
                        t = tmp.tile([128, TOK], F32, tag="lnt")
                        nc.scalar.activation(t[:], ps[:], AF.Identity,
                                             bias=bias_a[:, l, NM_QKV + m:NM_QKV + m + 1])
                        nc.vector.tensor_add(x_fm[:, m, :], x_fm[:, m, :], t[:])

                _ln(h_bf)

                with tc.tile_pool(name="wf1", bufs=3) as pw:
                    wf = []
                    for j in range(3):
                        w = pw.tile([128, 2, HID], BF16, tag="w")
                        nc.sync.dma_start(
                            w[:],
                            d_f1w.ap()[l, 2 * j:2 * j + 2].rearrange("k p m -> p k m"))
                        wf.append(w)
                    wt = [(wf[k // 2], k % 2) for k in range(NK)]
                    for m in range(NM_HID):
                        ps = gemm_ps(m)
                        mm_block(ps, wt, m, h_bf, NK)
                        nc.scalar.activation(h2_sb[:, m, :], ps[:], AF.Gelu,
                                             bias=bias_a[:, l, 24 + m:24 + m + 1])

                with tc.tile_pool(name="wf2", bufs=4) as pw:
                    wf = []
                    for j in range(4):
                        w = pw.tile([128, 6, D], BF16, tag="w")
                        nc.sync.dma_start(
                            w[:],
                            d_f2w.ap()[l, 6 * j:6 * j + 6].rearrange("k p m -> p k m"))
                        wf.append(w)
                    wt = [(wf[k // 6], k % 6) for k in range(NM_HID)]
                    for m in range(NK):
                        ps = gemm_ps(m)
                        mm_block(ps, wt, m, h2_sb, NM_HID)
                        t = tmp.tile([128, TOK], F32, tag="lnt")
                        nc.scalar.activation(t[:], ps[:], AF.Identity,
                                             bias=bias_a[:, l, 48 + m:48 + m + 1])
                        nc.vector.tensor_add(x_fm[:, m, :], x_fm[:, m, :], t[:])

            # final LN + lm_head
            _ln(h_bf)

            with tc.tile_pool(name="wlm", bufs=6) as pw:
                for b in range(NVB):
                    w = pw.tile([128, NK, VB], BF16, tag="w")
                    nc.sync.dma_start(w[:], d_wemb.ap()[b])
                    ot = lmo.tile([128, 2, VB], BF16, tag="ot")
                    for qt in range(2):
                        if qt == 0:
                            ps = ps_mm.tile([128, VB], F32, tag="mm", name="lmps")
                        else:
                            ps = ps_q.tile([128, VB], F32, tag="sc", name="lmps")
                        for k in range(NK):
                            nc.tensor.matmul(ps[:],
                                             h_bf[:, k, 128 * qt:128 * (qt + 1)],
                                             w[:, k, :],
                                             start=(k == 0), stop=(k == NK - 1))
                        nc.vector.tensor_copy(ot[:, qt, :], ps[:])
                    nc.sync.dma_start(d_out.ap()[:, :, b, :], ot[:])

    nc.compile()
    return nc


def _prep_inputs(W_emb, pos_emb, norm1_g, norm1_b, qkv_w, qkv_b, proj_w, proj_b,
                 norm2_g, norm2_b, ffn_w1, ffn_b1, ffn_w2, ffn_b2, fin_g, fin_b,
                 input_ids):
    bf = ml_dtypes.bfloat16
    f32 = np.float32

    W_emb = np.asarray(W_emb, f32)
    pos_emb = np.asarray(pos_emb, f32)
    qkv_w = np.asarray(qkv_w, f32)
    qkv_b = np.asarray(qkv_b, f32)
    proj_w = np.asarray(proj_w, f32)
    proj_b = np.asarray(proj_b, f32)
    ffn_w1 = np.asarray(ffn_w1, f32)
    ffn_b1 = np.asarray(ffn_b1, f32)
    ffn_w2 = np.asarray(ffn_w2, f32)
    ffn_b2 = np.asarray(ffn_b2, f32)
    n1g, n1b = np.asarray(norm1_g, f32), np.asarray(norm1_b, f32)
    n2g, n2b = np.asarray(norm2_g, f32), np.asarray(norm2_b, f32)
    fin_g, fin_b = np.asarray(fin_g, f32), np.asarray(fin_b, f32)

    # Fold LN affines into the consuming weights
    qkv_w_eff = qkv_w * n1g[:, None, :]                       # [L,3D,D]
    qkv_b_eff = qkv_b + np.einsum("lod,ld->lo", qkv_w, n1b)
    f1w_eff = ffn_w1 * n2g[:, None, :]
    f1b_eff = ffn_b1 + np.einsum("lod,ld->lo", ffn_w1, n2b)
    wemb_eff = W_emb * fin_g[None, :]                         # lm_head side only
    lm_bias = W_emb @ fin_b                                   # [V], host-added

    # head-permute qkv to [q(all heads) | k | v], fold 1/sqrt(HD) into q
    qkv_r = qkv_w_eff.reshape(L, H, 3, HD, D).transpose(0, 2, 1, 3, 4) \
        .reshape(L, 3 * D, D).copy()
    qkv_b_r = qkv_b_eff.reshape(L, H, 3, HD).transpose(0, 2, 1, 3) \
        .reshape(L, 3 * D).copy()
    sc = 1.0 / math.sqrt(HD)
    qkv_r[:, :D, :] *= sc
    qkv_b_r[:, :D] *= sc

    def tp(a):  # [L, out, in] -> [L, NK, 128, out] bf16
        out_dim = a.shape[1]
        return np.ascontiguousarray(
            a.transpose(0, 2, 1).reshape(L, NK, 128, out_dim)).astype(bf)

    def btile(a, nm):  # [L, nm*128] -> [L, 128, nm]
        return np.ascontiguousarray(a.reshape(L, nm, 128).transpose(0, 2, 1))

    f2w_t = np.ascontiguousarray(
        ffn_w2.transpose(0, 2, 1).reshape(L, NM_HID, 128, D)).astype(bf)

    biases = np.concatenate([
        btile(qkv_b_r, NM_QKV), btile(proj_b, NK),
        btile(f1b_eff, NM_HID), btile(ffn_b2, NK)], axis=2)   # [L,128,54]
    biases = np.ascontiguousarray(biases.transpose(1, 0, 2))  # [128,L,54]

    wemb_t = np.ascontiguousarray(
        wemb_eff.T.reshape(NK, 128, NVB, VB).transpose(2, 1, 0, 3)).astype(bf)

    ids = np.asarray(input_ids).reshape(-1).astype(np.int64)
    x0 = W_emb[ids] * math.sqrt(D)
    x0 = x0 + pos_emb[np.tile(np.arange(S), B)]

    common = {
        "qkvw": tp(qkv_r),
        "projw": tp(proj_w),
        "f1w": tp(f1w_eff),
        "f2w": f2w_t,
        "biases": biases,
        "wemb": wemb_t,
    }

    kg = np.arange(4 * TOK)
    in_maps = []
    for c in range(N_CORES):
        # [128, NK, TOK]: element [p, k, t] = x0[t, k*128+p]
        xs = np.ascontiguousarray(
            x0[TOK * c:TOK * (c + 1)].T.reshape(NK, 128, TOK).transpose(1, 0, 2))
        p = c % 4
        qg = p * TOK + np.arange(TOK)
        m = np.where(qg[None, :] >= kg[:, None], 1.0, 0.0)
        m = np.ascontiguousarray(m.reshape(8, 128, TOK).transpose(1, 0, 2)).astype(bf)
        in_maps.append({"x0": xs.astype(f32), "mask": m, **common})
    return in_maps, lm_bias


def kernel(**inputs):
    global LAST_RESULT, _NC_CACHE
    in_maps, lm_bias = _prep_inputs(**inputs)
    if _NC_CACHE is None:
        _NC_CACHE = build_nc()
    res = run_bass_kernel_spmd(_NC_CACHE, in_maps, list(range(N_CORES)),
                               trace=TRACE)
    LAST_RESULT = res
    outs = []
    for c in range(N_CORES):
        o = np.asarray(res.results[c]["logits"]).astype(np.float32)
        # [128, 2, NVB, VB] -> [256, V]
        outs.append(o.transpose(1, 0, 2, 3).reshape(TOK, V))
    logits = np.concatenate(outs, axis=0).reshape(B, S, V)
    return (logits + lm_bias[None, None, :]).astype(np.float32)


# revision 14
# speedup vs baseline: 1.5840x; 1.1070x over previous
"""GPT decoder (V=32000,S=1024,D=768,H=12,HID=3072,L=4,B=2) on 8 trn2 cores.

Sharding: sequence-parallel body — core c owns tokens [256c, 256c+256) of the
flattened [2048] token stream (cores 0-3 = batch 0, cores 4-7 = batch 1).
Per layer K (split in two) then V are exchanged with pipelined AllGathers
inside each 4-core batch group; score matmuls (K-only) run while the V
exchange flies, AV runs after, so the in-order PE queue never blocks on a
collective.  The tied lm_head runs per-core over the full vocab for the
local 256 tokens.  LayerNorm affine params are folded into the adjacent
weights host-side; softmax masking is multiplicative (0/1) after exp.
Matmuls in bf16 with fp32 PSUM accumulation; logits stored bf16.
Activations are feature-major [D, tok] so contractions sit on partitions.
"""
import math

import ml_dtypes
import numpy as np

import concourse.bass as bass
import concourse.mybir as mybir
import concourse.tile as tile
from concourse import bacc
from concourse.bass_utils import run_bass_kernel_spmd

F32 = mybir.dt.float32
BF16 = mybir.dt.bfloat16
AF = mybir.ActivationFunctionType
ALU = mybir.AluOpType

N_CORES = 8
GROUPS = [[0, 1, 2, 3], [4, 5, 6, 7]]
V, S, D, H, HID, L, B = 32000, 1024, 768, 12, 3072, 4, 2
HD = D // H          # 64
TOK = 256            # tokens per core
NK = D // 128        # 6 feature chunks
NM_QKV = 3 * D // 128   # 18
NM_HID = HID // 128     # 24
EPS = 1e-5
VB = 500             # lm_head vocab block
NVB = V // VB        # 64
NBIAS = NM_QKV + NK + NM_HID + NK   # 54 bias cols per layer

TRACE = False
LAST_RESULT = None

_NC_CACHE = None


def build_nc():
    nc = bacc.Bacc("TRN2", target_bir_lowering=False, debug=False,
                   enable_asserts=True, num_devices=N_CORES)

    d_x0 = nc.dram_tensor("x0", [128, NK, TOK], F32, kind="ExternalInput")
    d_mask = nc.dram_tensor("mask", [128, 8, TOK], BF16, kind="ExternalInput")
    d_bias = nc.dram_tensor("biases", [128, L, NBIAS], F32, kind="ExternalInput")
    d_qkvw = nc.dram_tensor("qkvw", [L, NK, 128, 3 * D], BF16, kind="ExternalInput")
    d_projw = nc.dram_tensor("projw", [L, NK, 128, D], BF16, kind="ExternalInput")
    d_f1w = nc.dram_tensor("f1w", [L, NK, 128, HID], BF16, kind="ExternalInput")
    d_f2w = nc.dram_tensor("f2w", [L, NM_HID, 128, D], BF16, kind="ExternalInput")
    d_wemb = nc.dram_tensor("wemb", [NVB, 128, NK, VB], BF16, kind="ExternalInput")
    d_out = nc.dram_tensor("logits", [128, 2, NVB, VB], BF16, kind="ExternalOutput")

    with tile.TileContext(nc) as tc:
        from contextlib import ExitStack
        with ExitStack() as ctx:
            const = ctx.enter_context(tc.tile_pool(name="const", bufs=1))
            res = ctx.enter_context(tc.tile_pool(name="res", bufs=1))
            tmp = ctx.enter_context(tc.tile_pool(name="tmp", bufs=4))
            lmo = ctx.enter_context(tc.tile_pool(name="lmo", bufs=6))
            ps_mm = ctx.enter_context(tc.tile_pool(name="ps_mm", bufs=2, space="PSUM"))
            ps_q = ctx.enter_context(tc.tile_pool(name="ps_q", bufs=2, space="PSUM"))
            ps_stat = ctx.enter_context(tc.tile_pool(name="ps_stat", bufs=2, space="PSUM"))
            dram = ctx.enter_context(tc.tile_pool(name="dram", bufs=2, space="DRAM"))

            identity = const.tile([128, 128], BF16)
            from concourse.masks import make_identity
            make_identity(nc, identity[:])
            ones_bf = const.tile([128, 1], BF16)
            nc.any.memset(ones_bf[:], 1.0)
            invD_row = const.tile([1, 128], BF16)
            nc.any.memset(invD_row[:], 1.0 / D)
            ones2 = const.tile([1, 64], F32)
            nc.any.memset(ones2[:], 1.0)
            eps_col = const.tile([128, 1], F32)
            nc.any.memset(eps_col[:], EPS)

            # Residual stream + mask + biases, resident in SBUF
            x_fm = res.tile([128, NK, TOK], F32)
            nc.sync.dma_start(x_fm[:], d_x0.ap())
            mask_t = res.tile([128, 8, TOK], BF16)
            nc.sync.dma_start(mask_t[:], d_mask.ap())
            bias_a = res.tile([128, L, NBIAS], F32)
            nc.sync.dma_start(bias_a[:], d_bias.ap())

            h_bf = res.tile([128, NK, TOK], BF16)
            xsq = res.tile([128, 2, NK, TOK], BF16)   # LN stats scratch
            qkv_sb = res.tile([128, NM_QKV, TOK], BF16)
            # v_own: per head 66 cols = [ones | v(64) | ones] (264-col slabs)
            v_own = res.tile([128, 2, H, 66], BF16)
            nc.any.memset(v_own[:, :, :, 0:1], 1.0)
            nc.any.memset(v_own[:, :, :, 65:66], 1.0)
            k_lo = res.tile([128, 4, 3, TOK], BF16)   # [p, rank, kchunk 0-2, t]
            k_hi = res.tile([128, 4, 3, TOK], BF16)   # [p, rank, kchunk 3-5, t]
            v_all = res.tile([128, 4, 2, H, 66], BF16)
            pt_all = res.tile([128, H, 8, TOK], BF16)  # exp'd masked scores
            vals_fm = res.tile([128, NK, TOK], BF16)
            h2_sb = res.tile([128, NM_HID, TOK], BF16)

            def _ln(out_bf):
                """out_bf = (x - mean)/sqrt(var+eps) over features."""
                for k in range(NK):
                    nc.vector.tensor_copy(xsq[:, 0, k, :], x_fm[:, k, :])
                    nc.vector.tensor_mul(xsq[:, 1, k, :], xsq[:, 0, k, :],
                                         xsq[:, 0, k, :])
                s1 = ps_stat.tile([1, TOK], F32, tag="stat")
                s2 = ps_stat.tile([1, TOK], F32, tag="stat")
                for k in range(NK):
                    nc.tensor.matmul(s1[:], ones_bf[:], xsq[:, 0, k, :],
                                     start=(k == 0), stop=(k == NK - 1))
                    nc.tensor.matmul(s2[:], ones_bf[:], xsq[:, 1, k, :],
                                     start=(k == 0), stop=(k == NK - 1))
                s12 = tmp.tile([1, 2, TOK], BF16, tag="s12")
                nc.vector.tensor_copy(s12[:, 0, :], s1[:])
                nc.vector.tensor_copy(s12[:, 1, :], s2[:])
                mean_bc = ps_mm.tile([128, TOK], F32, tag="mm")
                nc.tensor.matmul(mean_bc[:], invD_row[:], s12[:, 0, :],
                                 start=True, stop=True)
                m2_bc = ps_mm.tile([128, TOK], F32, tag="mm")
                nc.tensor.matmul(m2_bc[:], invD_row[:], s12[:, 1, :],
                                 start=True, stop=True)
                msq = tmp.tile([128, TOK], F32, tag="lnf")
                nc.scalar.activation(msq[:], mean_bc[:], AF.Square)
                var = tmp.tile([128, TOK], F32, tag="lnf")
                nc.vector.tensor_sub(var[:], m2_bc[:], msq[:])
                sd = tmp.tile([128, TOK], F32, tag="lnf")
                nc.scalar.activation(sd[:], var[:], AF.Sqrt, bias=eps_col[:])
                rstd = tmp.tile([128, TOK], F32, tag="lnf")
                nc.vector.reciprocal(rstd[:], sd[:])
                for k in range(NK):
                    t = tmp.tile([128, TOK], F32, tag="lnt")
                    nc.vector.tensor_sub(t[:], x_fm[:, k, :], mean_bc[:])
                    nc.vector.tensor_mul(out_bf[:, k, :], t[:], rstd[:])

            def gemm_ps(i):
                """Alternate PSUM pools so 4 accumulation groups are in flight."""
                if i % 2 == 0:
                    return ps_mm.tile([128, TOK], F32, tag="mm", name="gps")
                return ps_q.tile([128, TOK], F32, tag="sc", name="gps")

            def mm_block(ps, wtiles, m, rhs_t, nk):
                for k in range(nk):
                    ti, j = wtiles[k]
                    nc.tensor.matmul(ps[:], ti[:, j, 128 * m:128 * (m + 1)],
                                     rhs_t[:, k, :], start=(k == 0), stop=(k == nk - 1))

            for l in range(L):
                _ln(h_bf)

                with tc.tile_pool(name="wqkv", bufs=3) as pw:
                    wk = []
                    for j in range(3):
                        w = pw.tile([128, 2, 3 * D], BF16, tag="w")
                        nc.sync.dma_start(
                            w[:],
                            d_qkvw.ap()[l, 2 * j:2 * j + 2].rearrange("k p m -> p k m"))
                        wk.append(w)
                    wt = [(wk[k // 2], k % 2) for k in range(NK)]

                    # K chunks first, in two halves, so gathers launch early
                    for i, m in enumerate(range(NK, 2 * NK)):
                        ps = gemm_ps(i)
                        mm_block(ps, wt, m, h_bf, NK)
                        nc.scalar.activation(qkv_sb[:, m, :], ps[:], AF.Identity,
                                             bias=bias_a[:, l, m:m + 1])
                        if m == NK + 2:
                            bK1_in = dram.tile([128, 3, TOK], BF16, tag="bk1i")
                            bK1_out = dram.tile([4, 128, 3, TOK], BF16, tag="bk1o")
                            nc.sync.dma_start(bK1_in[:], qkv_sb[:, NK:NK + 3, :])
                            nc.gpsimd.collective_compute(
                                "AllGather", ALU.bypass, replica_groups=GROUPS,
                                ins=[bK1_in.opt()], outs=[bK1_out.opt()])
                    bK2_in = dram.tile([128, 3, TOK], BF16, tag="bk2i")
                    bK2_out = dram.tile([4, 128, 3, TOK], BF16, tag="bk2o")
                    nc.sync.dma_start(bK2_in[:], qkv_sb[:, NK + 3:2 * NK, :])
                    nc.gpsimd.collective_compute(
                        "AllGather", ALU.bypass, replica_groups=GROUPS,
                        ins=[bK2_in.opt()], outs=[bK2_out.opt()])

                    # V chunks + transpose to token-major
                    for i, m in enumerate(range(2 * NK, 3 * NK)):
                        ps = gemm_ps(i)
                        mm_block(ps, wt, m, h_bf, NK)
                        nc.scalar.activation(qkv_sb[:, m, :], ps[:], AF.Identity,
                                             bias=bias_a[:, l, m:m + 1])
                    for h in range(H):
                        o = (h % 2) * 64
                        for t in range(2):
                            src = qkv_sb[o:o + 64, 12 + h // 2, 128 * t:128 * (t + 1)]
                            pt = ps_q.tile([128, 64], BF16, tag="sc")
                            nc.tensor.transpose(pt[:], src,
                                                identity[o:o + 64, o:o + 64])
                            nc.vector.tensor_copy(v_own[:, t, h, 1:65], pt[:])
                    bV_in = dram.tile([128, 2, H, 66], BF16, tag="bvin")
                    bV_out = dram.tile([4, 128, 2, H, 66], BF16, tag="bvout")
                    nc.sync.dma_start(bV_in[:], v_own[:])
                    nc.gpsimd.collective_compute(
                        "AllGather", ALU.bypass, replica_groups=GROUPS,
                        ins=[bV_in.opt()], outs=[bV_out.opt()])

                    # Q chunks (overlap the K gathers)
                    for i, m in enumerate(range(NK)):
                        ps = gemm_ps(i)
                        mm_block(ps, wt, m, h_bf, NK)
                        nc.scalar.activation(qkv_sb[:, m, :], ps[:], AF.Identity,
                                             bias=bias_a[:, l, m:m + 1])

                nc.sync.dma_start(k_lo[:], bK1_out[:].rearrange("c p k t -> p c k t"))
                nc.sync.dma_start(k_hi[:], bK2_out[:].rearrange("c p k t -> p c k t"))
                nc.sync.dma_start(v_all[:], bV_out[:].rearrange("c p t h x -> p c t h x"))

                # scores (K only) -> exp -> mask, all heads, before any AV
                dns = {}
                for h in range(H):
                    o = (h % 2) * 64
                    kslc = slice(o, o + 64)
                    kt = h // 2
                    ksrc = k_lo if kt < 3 else k_hi
                    kj = kt % 3
                    for half in range(2):
                        st = ps_q.tile([128, 4, TOK], F32, tag="sc")
                        for j in range(4):
                            kc = 4 * half + j
                            c, hf = kc // 2, kc % 2
                            nc.tensor.matmul(
                                st[:, j, :],
                                ksrc[kslc, c, kj, 128 * hf:128 * (hf + 1)],
                                qkv_sb[kslc, h // 2, :],
                                start=True, stop=True)
                        sl = pt_all[:, h, 4 * half:4 * half + 4, :]
                        nc.scalar.activation(sl, st[:], AF.Exp)
                        nc.vector.tensor_mul(sl, sl,
                                             mask_t[:, 4 * half:4 * half + 4, :])
                    dn = dns[h] = ps_stat.tile([1, TOK], F32, tag="stat", name="dn")
                    for kc in range(8):
                        nc.tensor.matmul(dn[:], ones_bf[:], pt_all[:, h, kc, :],
                                         start=(kc == 0), stop=(kc == 7))

                # AV + normalize (heads paired: one wide reciprocal per pair)
                for hp in range(H // 2):
                    avs = []
                    for h in (2 * hp, 2 * hp + 1):
                        o = (h % 2) * 64
                        av = ps_mm.tile([128, TOK], F32, tag="mm", name="av")
                        for kc in range(8):
                            c, hf = kc // 2, kc % 2
                            nc.tensor.matmul(av[o:o + 64, :],
                                             v_all[:, c, hf, h, 1:65],
                                             pt_all[:, h, kc, :],
                                             start=(kc == 0), stop=(kc == 7))
                        avs.append(av)
                    bc = ps_q.tile([128, TOK], F32, tag="sc")
                    for h in (2 * hp, 2 * hp + 1):
                        o = (h % 2) * 64
                        dn_sb = tmp.tile([1, TOK], F32, tag="dnsb")
                        nc.vector.tensor_copy(dn_sb[:], dns[h][:])
                        nc.tensor.matmul(bc[o:o + 64, :], ones2[:],
                                         dn_sb[:], start=True, stop=True)
                    rv = tmp.tile([128, TOK], F32, tag="rv")
                    nc.vector.reciprocal(rv[:], bc[:])
                    for h in (2 * hp, 2 * hp + 1):
                        o = (h % 2) * 64
                        nc.vector.tensor_mul(vals_fm[o:o + 64, hp, :],
                                             avs[h % 2][o:o + 64, :],
                                             rv[o:o + 64, :])

                # proj + residual
                with tc.tile_pool(name="wproj", bufs=2) as pw:
                    pk = []
                    for j in range(2):
                        w = pw.tile([128, 3, D], BF16, tag="w")
                        nc.sync.dma_start(
                            w[:],
                            d_projw.ap()[l, 3 * j:3 * j + 3].rearrange("k p m -> p k m"))
                        pk.append(w)
                    wt = [(pk[k // 3], k % 3) for k in range(NK)]
                    for m in range(NK):
                        ps = gemm_ps(m)
                        mm_block(ps, wt, m, vals_fm, NK)
                        t = tmp.tile([128, TOK], F32, tag="lnt")
                        nc.scalar.activation(t[:], ps[:], AF.Identity,
                                             bias=bias_a[:, l, NM_QKV + m:NM_QKV + m + 1])
                        nc.vector.tensor_add(x_fm[:, m, :], x_fm[:, m, :], t[:])

                _ln(h_bf)

                with tc.tile_pool(name="wf1", bufs=3) as pw:
                    wf = []
                    for j in range(3):
                        w = pw.tile([128, 2, HID], BF16, tag="w")
                        nc.sync.dma_start(
                            w[:],
                            d_f1w.ap()[l, 2 * j:2 * j + 2].rearrange("k p m -> p k m"))
                        wf.append(w)
                    wt = [(wf[k // 2], k % 2) for k in range(NK)]
                    for m in range(NM_HID):
                        ps = gemm_ps(m)
                        mm_block(ps, wt, m, h_bf, NK)
                        nc.scalar.activation(h2_sb[:, m, :], ps[:], AF.Gelu,
                                             bias=bias_a[:, l, 24 + m:24 + m + 1])

                with tc.tile_pool(name="wf2", bufs=4) as pw:
                    wf = []
                    for j in range(4):
                        w = pw.tile([128, 6, D], BF16, tag="w")
                        nc.sync.dma_start(
                            w[:],
                            d_f2w.ap()[l, 6 * j:6 * j + 6].rearrange("k p m -> p k m"))
                        wf.append(w)
                    wt = [(wf[k // 6], k % 6) for k in range(NM_HID)]
                    for m in range(NK):
                        ps = gemm_ps(m)
                        mm_block(ps, wt, m, h2_sb, NM_HID)
                        t = tmp.tile([128, TOK], F32, tag="lnt")
                        nc.scalar.activation(t[:], ps[:], AF.Identity,
                                             bias=bias_a[:, l, 48 + m:48 + m + 1])
                        nc.vector.tensor_add(x_fm[:, m, :], x_fm[:, m, :], t[:])

            # final LN + lm_head
            _ln(h_bf)

            with tc.tile_pool(name="wlm", bufs=6) as pw:
                for b in range(NVB):
                    w = pw.tile([128, NK, VB], BF16, tag="w")
                    nc.sync.dma_start(w[:], d_wemb.ap()[b])
                    ot = lmo.tile([128, 2, VB], BF16, tag="ot")
                    for qt in range(2):
                        if qt == 0:
                            ps = ps_mm.tile([128, VB], F32, tag="mm", name="lmps")
                        else:
                            ps = ps_q.tile([128, VB], F32, tag="sc", name="lmps")
                        for k in range(NK):
                            nc.tensor.matmul(ps[:],
                                             h_bf[:, k, 128 * qt:128 * (qt + 1)],
                                             w[:, k, :],
                                             start=(k == 0), stop=(k == NK - 1))
                        nc.vector.tensor_copy(ot[:, qt, :], ps[:])
                    nc.sync.dma_start(d_out.ap()[:, :, b, :], ot[:])

    nc.compile()
    return nc


def _prep_inputs(W_emb, pos_emb, norm1_g, norm1_b, qkv_w, qkv_b, proj_w, proj_b,
                 norm2_g, norm2_b, ffn_w1, ffn_b1, ffn_w2, ffn_b2, fin_g, fin_b,
                 input_ids):
    bf = ml_dtypes.bfloat16
    f32 = np.float32

    W_emb = np.asarray(W_emb, f32)
    pos_emb = np.asarray(pos_emb, f32)
    qkv_w = np.asarray(qkv_w, f32)
    qkv_b = np.asarray(qkv_b, f32)
    proj_w = np.asarray(proj_w, f32)
    proj_b = np.asarray(proj_b, f32)
    ffn_w1 = np.asarray(ffn_w1, f32)
    ffn_b1 = np.asarray(ffn_b1, f32)
    ffn_w2 = np.asarray(ffn_w2, f32)
    ffn_b2 = np.asarray(ffn_b2, f32)
    n1g, n1b = np.asarray(norm1_g, f32), np.asarray(norm1_b, f32)
    n2g, n2b = np.asarray(norm2_g, f32), np.asarray(norm2_b, f32)
    fin_g, fin_b = np.asarray(fin_g, f32), np.asarray(fin_b, f32)

    # Fold LN affines into the consuming weights
    qkv_w_eff = qkv_w * n1g[:, None, :]                       # [L,3D,D]
    qkv_b_eff = qkv_b + np.einsum("lod,ld->lo", qkv_w, n1b)
    f1w_eff = ffn_w1 * n2g[:, None, :]
    f1b_eff = ffn_b1 + np.einsum("lod,ld->lo", ffn_w1, n2b)
    wemb_eff = W_emb * fin_g[None, :]                         # lm_head side only
    lm_bias = W_emb @ fin_b                                   # [V], host-added

    # head-permute qkv to [q(all heads) | k | v], fold 1/sqrt(HD) into q
    qkv_r = qkv_w_eff.reshape(L, H, 3, HD, D).transpose(0, 2, 1, 3, 4) \
        .reshape(L, 3 * D, D).copy()
    qkv_b_r = qkv_b_eff.reshape(L, H, 3, HD).transpose(0, 2, 1, 3) \
        .reshape(L, 3 * D).copy()
    sc = 1.0 / math.sqrt(HD)
    qkv_r[:, :D, :] *= sc
    qkv_b_r[:, :D] *= sc

    def tp(a):  # [L, out, in] -> [L, NK, 128, out] bf16
        out_dim = a.shape[1]
        return np.ascontiguousarray(
            a.transpose(0, 2, 1).reshape(L, NK, 128, out_dim)).astype(bf)

    def btile(a, nm):  # [L, nm*128] -> [L, 128, nm]
        return np.ascontiguousarray(a.reshape(L, nm, 128).transpose(0, 2, 1))

    f2w_t = np.ascontiguousarray(
        ffn_w2.transpose(0, 2, 1).reshape(L, NM_HID, 128, D)).astype(bf)

    biases = np.concatenate([
        btile(qkv_b_r, NM_QKV), btile(proj_b, NK),
        btile(f1b_eff, NM_HID), btile(ffn_b2, NK)], axis=2)   # [L,128,54]
    biases = np.ascontiguousarray(biases.transpose(1, 0, 2))  # [128,L,54]

    wemb_t = np.ascontiguousarray(
        wemb_eff.T.reshape(NK, 128, NVB, VB).transpose(2, 1, 0, 3)).astype(bf)

    ids = np.asarray(input_ids).reshape(-1).astype(np.int64)
    x0 = W_emb[ids] * math.sqrt(D)
    x0 = x0 + pos_emb[np.tile(np.arange(S), B)]

    common = {
        "qkvw": tp(qkv_r),
        "projw": tp(proj_w),
        "f1w": tp(f1w_eff),
        "f2w": f2w_t,
        "biases": biases,
        "wemb": wemb_t,
    }

    kg = np.arange(4 * TOK)
    in_maps = []
    for c in range(N_CORES):
        # [128, NK, TOK]: element [p, k, t] = x0[t, k*128+p]
        xs = np.ascontiguousarray(
            x0[TOK * c:TOK * (c + 1)].T.reshape(NK, 128, TOK).transpose(1, 0, 2))
        p = c % 4
        qg = p * TOK + np.arange(TOK)
        m = np.where(qg[None, :] >= kg[:, None], 1.0, 0.0)
        m = np.ascontiguousarray(m.reshape(8, 128, TOK).transpose(1, 0, 2)).astype(bf)
        in_maps.append({"x0": xs.astype(f32), "mask": m, **common})
    return in_maps, lm_bias


def kernel(**inputs):
    global LAST_RESULT, _NC_CACHE
    in_maps, lm_bias = _prep_inputs(**inputs)
    if _NC_CACHE is None:
        _NC_CACHE = build_nc()
    res = run_bass_kernel_spmd(_NC_CACHE, in_maps, list(range(N_CORES)),
                               trace=TRACE)
    LAST_RESULT = res
    outs = []
    for c in range(N_CORES):
        o = np.asarray(res.results[c]["logits"]).astype(np.float32)
        # [128, 2, NVB, VB] -> [256, V]
        outs.append(o.transpose(1, 0, 2, 3).reshape(TOK, V))
    logits = np.concatenate(outs, axis=0).reshape(B, S, V)
    return (logits + lm_bias[None, None, :]).astype(np.float32)


# revision 15
# speedup vs baseline: 1.6022x; 1.0115x over previous
"""GPT decoder (V=32000,S=1024,D=768,H=12,HID=3072,L=4,B=2) on 8 trn2 cores.

Sharding: sequence-parallel body — core c owns tokens [256c, 256c+256) of the
flattened [2048] token stream (cores 0-3 = batch 0, cores 4-7 = batch 1).
Per layer K (split in two) then V are exchanged with pipelined AllGathers
inside each 4-core batch group; score matmuls (K-only) run while the V
exchange flies, AV runs after, so the in-order PE queue never blocks on a
collective.  The tied lm_head runs per-core over the full vocab for the
local 256 tokens.  LayerNorm affine params are folded into the adjacent
weights host-side; softmax masking is multiplicative (0/1) after exp.
Matmuls in bf16 with fp32 PSUM accumulation; logits stored bf16.
Activations are feature-major [D, tok] so contractions sit on partitions.
"""
import math

import ml_dtypes
import numpy as np

import concourse.bass as bass
import concourse.mybir as mybir
import concourse.tile as tile
from concourse import bacc
from concourse.bass_utils import run_bass_kernel_spmd

F32 = mybir.dt.float32
BF16 = mybir.dt.bfloat16
AF = mybir.ActivationFunctionType
ALU = mybir.AluOpType

N_CORES = 8
GROUPS = [[0, 1, 2, 3], [4, 5, 6, 7]]
V, S, D, H, HID, L, B = 32000, 1024, 768, 12, 3072, 4, 2
HD = D // H          # 64
TOK = 256            # tokens per core
NK = D // 128        # 6 feature chunks
NM_QKV = 3 * D // 128   # 18
NM_HID = HID // 128     # 24
EPS = 1e-5
VB = 500             # lm_head vocab block
NVB = V // VB        # 64
NBIAS = NM_QKV + NK + NM_HID + NK   # 54 bias cols per layer

TRACE = False
LAST_RESULT = None

_NC_CACHE = None


def build_nc():
    nc = bacc.Bacc("TRN2", target_bir_lowering=False, debug=False,
                   enable_asserts=True, num_devices=N_CORES)

    d_x0 = nc.dram_tensor("x0", [128, NK, TOK], F32, kind="ExternalInput")
    d_mask = nc.dram_tensor("mask", [128, 8, TOK], BF16, kind="ExternalInput")
    d_bias = nc.dram_tensor("biases", [128, L, NBIAS], F32, kind="ExternalInput")
    d_qkvw = nc.dram_tensor("qkvw", [L, NK, 128, 3 * D], BF16, kind="ExternalInput")
    d_projw = nc.dram_tensor("projw", [L, NK, 128, D], BF16, kind="ExternalInput")
    d_f1w = nc.dram_tensor("f1w", [L, NK, 128, HID], BF16, kind="ExternalInput")
    d_f2w = nc.dram_tensor("f2w", [L, NM_HID, 128, D], BF16, kind="ExternalInput")
    d_wemb = nc.dram_tensor("wemb", [NVB, 128, NK, VB], BF16, kind="ExternalInput")
    d_out = nc.dram_tensor("logits", [128, 2, NVB, VB], BF16, kind="ExternalOutput")

    with tile.TileContext(nc) as tc:
        from contextlib import ExitStack
        with ExitStack() as ctx:
            const = ctx.enter_context(tc.tile_pool(name="const", bufs=1))
            res = ctx.enter_context(tc.tile_pool(name="res", bufs=1))
            tmp = ctx.enter_context(tc.tile_pool(name="tmp", bufs=4))
            lmo = ctx.enter_context(tc.tile_pool(name="lmo", bufs=8))
            ps_mm = ctx.enter_context(tc.tile_pool(name="ps_mm", bufs=2, space="PSUM"))
            ps_q = ctx.enter_context(tc.tile_pool(name="ps_q", bufs=2, space="PSUM"))
            ps_stat = ctx.enter_context(tc.tile_pool(name="ps_stat", bufs=2, space="PSUM"))
            dram = ctx.enter_context(tc.tile_pool(name="dram", bufs=3, space="DRAM"))

            identity = const.tile([128, 128], BF16)
            from concourse.masks import make_identity
            make_identity(nc, identity[:])
            ones_bf = const.tile([128, 1], BF16)
            nc.any.memset(ones_bf[:], 1.0)
            invD_row = const.tile([1, 128], BF16)
            nc.any.memset(invD_row[:], 1.0 / D)
            ones2 = const.tile([1, 64], F32)
            nc.any.memset(ones2[:], 1.0)
            eps_col = const.tile([128, 1], F32)
            nc.any.memset(eps_col[:], EPS)

            # Residual stream + mask + biases, resident in SBUF
            x_fm = res.tile([128, NK, TOK], F32)
            nc.sync.dma_start(x_fm[:], d_x0.ap())
            mask_t = res.tile([128, 8, TOK], BF16)
            nc.sync.dma_start(mask_t[:], d_mask.ap())
            bias_a = res.tile([128, L, NBIAS], F32)
            nc.sync.dma_start(bias_a[:], d_bias.ap())

            h_bf = res.tile([128, NK, TOK], BF16)
            xsq = res.tile([128, 2, NK, TOK], BF16)   # LN stats scratch
            qkv_sb = res.tile([128, NM_QKV, TOK], BF16)
            # v_own: per head 66 cols = [ones | v(64) | ones] (264-col slabs)
            v_own = res.tile([128, 2, H, 66], BF16)
            nc.any.memset(v_own[:, :, :, 0:1], 1.0)
            nc.any.memset(v_own[:, :, :, 65:66], 1.0)
            k_lo = res.tile([128, 4, 3, TOK], BF16)   # [p, rank, kchunk 0-2, t]
            k_hi = res.tile([128, 4, 3, TOK], BF16)   # [p, rank, kchunk 3-5, t]
            v_all = res.tile([128, 4, 2, H, 66], BF16)
            pt_all = res.tile([128, H, 8, TOK], BF16)  # exp'd masked scores
            vals_fm = res.tile([128, NK, TOK], BF16)
            h2_sb = res.tile([128, NM_HID, TOK], BF16)

            def _ln(out_bf):
                """out_bf = (x - mean)/sqrt(var+eps) over features."""
                for k in range(NK):
                    nc.vector.tensor_copy(xsq[:, 0, k, :], x_fm[:, k, :])
                    nc.vector.tensor_mul(xsq[:, 1, k, :], xsq[:, 0, k, :],
                                         xsq[:, 0, k, :])
                s1 = ps_stat.tile([1, TOK], F32, tag="stat")
                s2 = ps_stat.tile([1, TOK], F32, tag="stat")
                for k in range(NK):
                    nc.tensor.matmul(s1[:], ones_bf[:], xsq[:, 0, k, :],
                                     start=(k == 0), stop=(k == NK - 1))
                    nc.tensor.matmul(s2[:], ones_bf[:], xsq[:, 1, k, :],
                                     start=(k == 0), stop=(k == NK - 1))
                s12 = tmp.tile([1, 2, TOK], BF16, tag="s12")
                nc.vector.tensor_copy(s12[:, 0, :], s1[:])
                nc.vector.tensor_copy(s12[:, 1, :], s2[:])
                mean_bc = ps_mm.tile([128, TOK], F32, tag="mm")
                nc.tensor.matmul(mean_bc[:], invD_row[:], s12[:, 0, :],
                                 start=True, stop=True)
                m2_bc = ps_mm.tile([128, TOK], F32, tag="mm")
                nc.tensor.matmul(m2_bc[:], invD_row[:], s12[:, 1, :],
                                 start=True, stop=True)
                msq = tmp.tile([128, TOK], F32, tag="lnf")
                nc.scalar.activation(msq[:], mean_bc[:], AF.Square)
                var = tmp.tile([128, TOK], F32, tag="lnf")
                nc.vector.tensor_sub(var[:], m2_bc[:], msq[:])
                sd = tmp.tile([128, TOK], F32, tag="lnf")
                nc.scalar.activation(sd[:], var[:], AF.Sqrt, bias=eps_col[:])
                rstd = tmp.tile([128, TOK], F32, tag="lnf")
                nc.vector.reciprocal(rstd[:], sd[:])
                for k in range(NK):
                    t = tmp.tile([128, TOK], F32, tag="lnt")
                    nc.vector.tensor_sub(t[:], x_fm[:, k, :], mean_bc[:])
                    nc.vector.tensor_mul(out_bf[:, k, :], t[:], rstd[:])

            def gemm_ps(i):
                """Alternate PSUM pools so 4 accumulation groups are in flight."""
                if i % 2 == 0:
                    return ps_mm.tile([128, TOK], F32, tag="mm", name="gps")
                return ps_q.tile([128, TOK], F32, tag="sc", name="gps")

            def mm_block(ps, wtiles, m, rhs_t, nk):
                for k in range(nk):
                    ti, j = wtiles[k]
                    nc.tensor.matmul(ps[:], ti[:, j, 128 * m:128 * (m + 1)],
                                     rhs_t[:, k, :], start=(k == 0), stop=(k == nk - 1))

            for l in range(L):
                _ln(h_bf)

                with tc.tile_pool(name="wqkv", bufs=3) as pw:
                    wk = []
                    for j in range(3):
                        w = pw.tile([128, 2, 3 * D], BF16, tag="w")
                        nc.sync.dma_start(
                            w[:],
                            d_qkvw.ap()[l, 2 * j:2 * j + 2].rearrange("k p m -> p k m"))
                        wk.append(w)
                    wt = [(wk[k // 2], k % 2) for k in range(NK)]

                    # K chunks first, in two halves, so gathers launch early
                    for i, m in enumerate(range(NK, 2 * NK)):
                        ps = gemm_ps(i)
                        mm_block(ps, wt, m, h_bf, NK)
                        nc.scalar.activation(qkv_sb[:, m, :], ps[:], AF.Identity,
                                             bias=bias_a[:, l, m:m + 1])
                        if m == NK + 2:
                            bK1_in = dram.tile([128, 3, TOK], BF16, tag="bk1i")
                            bK1_out = dram.tile([4, 128, 3, TOK], BF16, tag="bk1o")
                            nc.sync.dma_start(bK1_in[:], qkv_sb[:, NK:NK + 3, :])
                            nc.gpsimd.collective_compute(
                                "AllGather", ALU.bypass, replica_groups=GROUPS,
                                ins=[bK1_in.opt()], outs=[bK1_out.opt()])
                    bK2_in = dram.tile([128, 3, TOK], BF16, tag="bk2i")
                    bK2_out = dram.tile([4, 128, 3, TOK], BF16, tag="bk2o")
                    nc.sync.dma_start(bK2_in[:], qkv_sb[:, NK + 3:2 * NK, :])
                    nc.gpsimd.collective_compute(
                        "AllGather", ALU.bypass, replica_groups=GROUPS,
                        ins=[bK2_in.opt()], outs=[bK2_out.opt()])

                    # V chunks + transpose to token-major
                    for i, m in enumerate(range(2 * NK, 3 * NK)):
                        ps = gemm_ps(i)
                        mm_block(ps, wt, m, h_bf, NK)
                        nc.scalar.activation(qkv_sb[:, m, :], ps[:], AF.Identity,
                                             bias=bias_a[:, l, m:m + 1])
                    for h in range(H):
                        o = (h % 2) * 64
                        for t in range(2):
                            src = qkv_sb[o:o + 64, 12 + h // 2, 128 * t:128 * (t + 1)]
                            pt = ps_q.tile([128, 64], BF16, tag="sc")
                            nc.tensor.transpose(pt[:], src,
                                                identity[o:o + 64, o:o + 64])
                            nc.vector.tensor_copy(v_own[:, t, h, 1:65], pt[:])
                    bV_in = dram.tile([128, 2, H, 66], BF16, tag="bvin")
                    bV_out = dram.tile([4, 128, 2, H, 66], BF16, tag="bvout")
                    nc.sync.dma_start(bV_in[:], v_own[:])
                    nc.gpsimd.collective_compute(
                        "AllGather", ALU.bypass, replica_groups=GROUPS,
                        ins=[bV_in.opt()], outs=[bV_out.opt()])

                    # Q chunks (overlap the K gathers)
                    for i, m in enumerate(range(NK)):
                        ps = gemm_ps(i)
                        mm_block(ps, wt, m, h_bf, NK)
                        nc.scalar.activation(qkv_sb[:, m, :], ps[:], AF.Identity,
                                             bias=bias_a[:, l, m:m + 1])

                nc.sync.dma_start(k_lo[:], bK1_out[:].rearrange("c p k t -> p c k t"))
                nc.sync.dma_start(k_hi[:], bK2_out[:].rearrange("c p k t -> p c k t"))
                nc.sync.dma_start(v_all[:], bV_out[:].rearrange("c p t h x -> p c t h x"))

                # scores (K only) -> exp -> mask, all heads, before any AV
                dns = {}
                for h in range(H):
                    o = (h % 2) * 64
                    kslc = slice(o, o + 64)
                    kt = h // 2
                    ksrc = k_lo if kt < 3 else k_hi
                    kj = kt % 3
                    for half in range(2):
                        st = ps_q.tile([128, 4, TOK], F32, tag="sc")
                        for j in range(4):
                            kc = 4 * half + j
                            c, hf = kc // 2, kc % 2
                            nc.tensor.matmul(
                                st[:, j, :],
                                ksrc[kslc, c, kj, 128 * hf:128 * (hf + 1)],
                                qkv_sb[kslc, h // 2, :],
                                start=True, stop=True)
                        sl = pt_all[:, h, 4 * half:4 * half + 4, :]
                        nc.scalar.activation(sl, st[:], AF.Exp)
                        nc.vector.tensor_mul(sl, sl,
                                             mask_t[:, 4 * half:4 * half + 4, :])
                    dn = dns[h] = ps_stat.tile([1, TOK], F32, tag="stat", name="dn")
                    for kc in range(8):
                        nc.tensor.matmul(dn[:], ones_bf[:], pt_all[:, h, kc, :],
                                         start=(kc == 0), stop=(kc == 7))

                # AV + normalize (heads paired: one wide reciprocal per pair)
                for hp in range(H // 2):
                    avs = []
                    for h in (2 * hp, 2 * hp + 1):
                        o = (h % 2) * 64
                        av = ps_mm.tile([128, TOK], F32, tag="mm", name="av")
                        for kc in range(8):
                            c, hf = kc // 2, kc % 2
                            nc.tensor.matmul(av[o:o + 64, :],
                                             v_all[:, c, hf, h, 1:65],
                                             pt_all[:, h, kc, :],
                                             start=(kc == 0), stop=(kc == 7))
                        avs.append(av)
                    bc = ps_q.tile([128, TOK], F32, tag="sc")
                    for h in (2 * hp, 2 * hp + 1):
                        o = (h % 2) * 64
                        dn_sb = tmp.tile([1, TOK], F32, tag="dnsb")
                        nc.vector.tensor_copy(dn_sb[:], dns[h][:])
                        nc.tensor.matmul(bc[o:o + 64, :], ones2[:],
                                         dn_sb[:], start=True, stop=True)
                    rv = tmp.tile([128, TOK], F32, tag="rv")
                    nc.vector.reciprocal(rv[:], bc[:])
                    for h in (2 * hp, 2 * hp + 1):
                        o = (h % 2) * 64
                        nc.vector.tensor_mul(vals_fm[o:o + 64, hp, :],
                                             avs[h % 2][o:o + 64, :],
                                             rv[o:o + 64, :])

                # proj + residual
                with tc.tile_pool(name="wproj", bufs=2) as pw:
                    pk = []
                    for j in range(2):
                        w = pw.tile([128, 3, D], BF16, tag="w")
                        nc.sync.dma_start(
                            w[:],
                            d_projw.ap()[l, 3 * j:3 * j + 3].rearrange("k p m -> p k m"))
                        pk.append(w)
                    wt = [(pk[k // 3], k % 3) for k in range(NK)]
                    for m in range(NK):
                        ps = gemm_ps(m)
                        mm_block(ps, wt, m, vals_fm, NK)
                        t = tmp.tile([128, TOK], F32, tag="lnt")
                        nc.scalar.activation(t[:], ps[:], AF.Identity,
                                             bias=bias_a[:, l, NM_QKV + m:NM_QKV + m + 1])
                        nc.vector.tensor_add(x_fm[:, m, :], x_fm[:, m, :], t[:])

                _ln(h_bf)

                with tc.tile_pool(name="wf1", bufs=3) as pw:
                    wf = []
                    for j in range(3):
                        w = pw.tile([128, 2, HID], BF16, tag="w")
                        nc.sync.dma_start(
                            w[:],
                            d_f1w.ap()[l, 2 * j:2 * j + 2].rearrange("k p m -> p k m"))
                        wf.append(w)
                    wt = [(wf[k // 2], k % 2) for k in range(NK)]
                    for m in range(NM_HID):
                        ps = gemm_ps(m)
                        mm_block(ps, wt, m, h_bf, NK)
                        nc.scalar.activation(h2_sb[:, m, :], ps[:], AF.Gelu,
                                             bias=bias_a[:, l, 24 + m:24 + m + 1])

                with tc.tile_pool(name="wf2", bufs=5) as pw:
                    wf = []
                    for j in range(4):
                        w = pw.tile([128, 6, D], BF16, tag="w")
                        nc.sync.dma_start(
                            w[:],
                            d_f2w.ap()[l, 6 * j:6 * j + 6].rearrange("k p m -> p k m"))
                        wf.append(w)
                    wt = [(wf[k // 6], k % 6) for k in range(NM_HID)]
                    for m in range(NK):
                        ps = gemm_ps(m)
                        mm_block(ps, wt, m, h2_sb, NM_HID)
                        t = tmp.tile([128, TOK], F32, tag="lnt")
                        nc.scalar.activation(t[:], ps[:], AF.Identity,
                                             bias=bias_a[:, l, 48 + m:48 + m + 1])
                        nc.vector.tensor_add(x_fm[:, m, :], x_fm[:, m, :], t[:])

            # final LN + lm_head
            _ln(h_bf)

            with tc.tile_pool(name="wlm", bufs=8) as pw:
                for b in range(NVB):
                    w = pw.tile([128, NK, VB], BF16, tag="w")
                    nc.sync.dma_start(w[:], d_wemb.ap()[b])
                    ot = lmo.tile([128, 2, VB], BF16, tag="ot")
                    for qt in range(2):
                        if qt == 0:
                            ps = ps_mm.tile([128, VB], F32, tag="mm", name="lmps")
                        else:
                            ps = ps_q.tile([128, VB], F32, tag="sc", name="lmps")
                        for k in range(NK):
                            nc.tensor.matmul(ps[:],
                                             h_bf[:, k, 128 * qt:128 * (qt + 1)],
                                             w[:, k, :],
                                             start=(k == 0), stop=(k == NK - 1))
                        nc.vector.tensor_copy(ot[:, qt, :], ps[:])
                    nc.sync.dma_start(d_out.ap()[:, :, b, :], ot[:])

    nc.compile()
    return nc


def _prep_inputs(W_emb, pos_emb, norm1_g, norm1_b, qkv_w, qkv_b, proj_w, proj_b,
                 norm2_g, norm2_b, ffn_w1, ffn_b1, ffn_w2, ffn_b2, fin_g, fin_b,
                 input_ids):
    bf = ml_dtypes.bfloat16
    f32 = np.float32

    W_emb = np.asarray(W_emb, f32)
    pos_emb = np.asarray(pos_emb, f32)
    qkv_w = np.asarray(qkv_w, f32)
    qkv_b = np.asarray(qkv_b, f32)
    proj_w = np.asarray(proj_w, f32)
    proj_b = np.asarray(proj_b, f32)
    ffn_w1 = np.asarray(ffn_w1, f32)
    ffn_b1 = np.asarray(ffn_b1, f32)
    ffn_w2 = np.asarray(ffn_w2, f32)
    ffn_b2 = np.asarray(ffn_b2, f32)
    n1g, n1b = np.asarray(norm1_g, f32), np.asarray(norm1_b, f32)
    n2g, n2b = np.asarray(norm2_g, f32), np.asarray(norm2_b, f32)
    fin_g, fin_b = np.asarray(fin_g, f32), np.asarray(fin_b, f32)

    # Fold LN affines into the consuming weights
    qkv_w_eff = qkv_w * n1g[:, None, :]                       # [L,3D,D]
    qkv_b_eff = qkv_b + np.einsum("lod,ld->lo", qkv_w, n1b)
    f1w_eff = ffn_w1 * n2g[:, None, :]
    f1b_eff = ffn_b1 + np.einsum("lod,ld->lo", ffn_w1, n2b)
    wemb_eff = W_emb * fin_g[None, :]                         # lm_head side only
    lm_bias = W_emb @ fin_b                                   # [V], host-added

    # head-permute qkv to [q(all heads) | k | v], fold 1/sqrt(HD) into q
    qkv_r = qkv_w_eff.reshape(L, H, 3, HD, D).transpose(0, 2, 1, 3, 4) \
        .reshape(L, 3 * D, D).copy()
    qkv_b_r = qkv_b_eff.reshape(L, H, 3, HD).transpose(0, 2, 1, 3) \
        .reshape(L, 3 * D).copy()
    sc = 1.0 / math.sqrt(HD)
    qkv_r[:, :D, :] *= sc
    qkv_b_r[:, :D] *= sc

    def tp(a):  # [L, out, in] -> [L, NK, 128, out] bf16
        out_dim = a.shape[1]
        return np.ascontiguousarray(
            a.transpose(0, 2, 1).reshape(L, NK, 128, out_dim)).astype(bf)

    def btile(a, nm):  # [L, nm*128] -> [L, 128, nm]
        return np.ascontiguousarray(a.reshape(L, nm, 128).transpose(0, 2, 1))

    f2w_t = np.ascontiguousarray(
        ffn_w2.transpose(0, 2, 1).reshape(L, NM_HID, 128, D)).astype(bf)

    biases = np.concatenate([
        btile(qkv_b_r, NM_QKV), btile(proj_b, NK),
        btile(f1b_eff, NM_HID), btile(ffn_b2, NK)], axis=2)   # [L,128,54]
    biases = np.ascontiguousarray(biases.transpose(1, 0, 2))  # [128,L,54]

    wemb_t = np.ascontiguousarray(
        wemb_eff.T.reshape(NK, 128, NVB, VB).transpose(2, 1, 0, 3)).astype(bf)

    ids = np.asarray(input_ids).reshape(-1).astype(np.int64)
    x0 = W_emb[ids] * math.sqrt(D)
    x0 = x0 + pos_emb[np.tile(np.arange(S), B)]

    common = {
        "qkvw": tp(qkv_r),
        "projw": tp(proj_w),
        "f1w": tp(f1w_eff),
        "f2w": f2w_t,
        "biases": biases,
        "wemb": wemb_t,
    }

    kg = np.arange(4 * TOK)
    in_maps = []
    for c in range(N_CORES):
        # [128, NK, TOK]: element [p, k, t] = x0[t, k*128+p]
        xs = np.ascontiguousarray(
            x0[TOK * c:TOK * (c + 1)].T.reshape(NK, 128, TOK).transpose(1, 0, 2))
        p = c % 4
        qg = p * TOK + np.arange(TOK)
        m = np.where(qg[None, :] >= kg[:, None], 1.0, 0.0)
        m = np.ascontiguousarray(m.reshape(8, 128, TOK).transpose(1, 0, 2)).astype(bf)
        in_maps.append({"x0": xs.astype(f32), "mask": m, **common})
    return in_maps, lm_bias


def kernel(**inputs):
    global LAST_RESULT, _NC_CACHE
    in_maps, lm_bias = _prep_inputs(**inputs)
    if _NC_CACHE is None:
        _NC_CACHE = build_nc()
    res = run_bass_kernel_spmd(_NC_CACHE, in_maps, list(range(N_CORES)),
                               trace=TRACE)
    LAST_RESULT = res
    outs = []
    for c in range(N_CORES):
        o = np.asarray(res.results[c]["logits"]).astype(np.float32)
        # [128, 2, NVB, VB] -> [256, V]
        outs.append(o.transpose(1, 0, 2, 3).reshape(TOK, V))
    logits = np.concatenate(outs, axis=0).reshape(B, S, V)
    return (logits + lm_bias[None, None, :]).astype(np.float32)
